# revision 3
# baseline (speedup 1.0000x reference)
"""nn_GATv2Net kernel for 8 TRN2 NeuronCores (self-contained).

kernel(**inputs) takes the FULL unsharded inputs of the reference
(x [50000,1280], edge_index [2,800000] plus weights) and returns
(preds [50000], alpha0 [850000,4], alpha1 [850000,4]) as float32, matching
reference.reference().

Distribution: nodes block-partitioned across the 8 cores; each edge is owned
by the core holding its dst node. Edges are grouped into per-core blocks of
128 consecutive local dst nodes and sorted by src within the block. Per-edge
src features are fetched with SWDGE dma_gather from an AllGather-replicated
xl table; segment softmax + aggregation run as one-hot f32r matmuls in PSUM.
"""
import dataclasses
import numpy as np

import concourse.bacc as bacc
import concourse.mybir as mybir
from concourse import tile
from concourse.bass_utils import run_bass_kernel_spmd

F32 = mybir.dt.float32
F32R = mybir.dt.float32r
I16 = mybir.dt.int16
AF = mybir.ActivationFunctionType
ALU = mybir.AluOpType

NCORES = 8
N, E, IN_DIM, HID, H, D = 50000, 800000, 1280, 256, 4, 64
NEG = 0.2


@dataclasses.dataclass
class Cfg:
    N: int = N
    E: int = E
    IN_DIM: int = IN_DIM
    HID: int = HID
    H: int = H
    D: int = D
    NEG: float = NEG
    NLOC: int = N // NCORES
    NB: int = (N // NCORES + 127) // 128
    NPAD: int = ((N // NCORES + 127) // 128) * 128
    NCH: int = 18
    CALL_CHUNKS: tuple = (8, 8, 2)

    @property
    def NGPAD(self):
        return NCORES * self.NPAD

    @property
    def KIN(self):
        return self.IN_DIM // 128


def make_cfg(N_, E_, IN_DIM_, nch, call_size=8):
    nloc = N_ // NCORES
    nb = (nloc + 127) // 128
    cc = []
    left = nch
    while left > 0:
        cc.append(min(call_size, left))
        left -= cc[-1]
    return Cfg(N=N_, E=E_, IN_DIM=IN_DIM_, NLOC=nloc, NB=nb, NPAD=nb * 128,
               NCH=nch, CALL_CHUNKS=tuple(cc))


# ---------------------------------------------------------------- host prep

def wrap_idx_flat(idx):
    """[n] int (n % 16 == 0) -> [128, n//16] int16 SWDGE wrapped layout."""
    n = idx.shape[0]
    a = idx.astype(np.int16).reshape(n // 16, 16)
    return np.tile(a.T, (8, 1))


def prep_graph(cfg, edge_index):
    src = np.asarray(edge_index[0], np.int64)
    dst = np.asarray(edge_index[1], np.int64)

    core = dst // cfg.NLOC
    ldst = dst - core * cfg.NLOC
    blk = ldst // 128
    drel = ldst % 128
    sg = (src // cfg.NLOC) * cfg.NPAD + (src % cfg.NLOC)

    order = np.lexsort((sg, blk, core))
    core_s, blk_s, drel_s, sg_s = core[order], blk[order], drel[order], sg[order]

    NCH, NB = cfg.NCH, cfg.NB
    cap = NCH * 128
    key = core_s * NB + blk_s
    bounds = np.searchsorted(key, np.arange(NCORES * NB + 1))
    counts = np.diff(bounds)
    if counts.max() > cap:
        raise ValueError(f"block overflow: {counts.max()} > {cap}")

    slot_sg = np.zeros((NCORES * NB, cap), np.int64)
    slot_dr = np.full((NCORES * NB, cap), -1.0, np.float32)
    slot_oid = np.full((NCORES * NB, cap), -1, np.int64)
    ar = np.arange(cap)
    mask = ar[None, :] < counts[:, None]
    idx_flat = np.nonzero(mask)
    slot_sg[idx_flat] = sg_s
    slot_dr[idx_flat] = drel_s.astype(np.float32)
    slot_oid[idx_flat] = order
    lastv = slot_sg[np.arange(NCORES * NB), np.maximum(counts - 1, 0)]
    slot_sg[~mask] = np.repeat(lastv, cap - counts)
    slot_sg[counts == 0] = 0

    slot_sg = slot_sg.reshape(NCORES, NB, NCH, 128)
    slot_dr = slot_dr.reshape(NCORES, NB, NCH, 128)
    slot_oid = slot_oid.reshape(NCORES, NB, NCH, 128)

    bases, calls = [], []
    c0 = 0
    for nch in cfg.CALL_CHUNKS:
        seg = slot_sg[:, :, c0:c0 + nch, :]
        b = int(seg.min())
        span = int(seg.max()) - b
        if span > 32767:
            raise ValueError(f"span {span} > 32767 (chunks {c0}..{c0+nch})")
        bases.append(b)
        calls.append((c0, nch))
        c0 += nch

    idx16 = np.zeros((NCORES, NB, 128, NCH * 8), np.int16)
    for (c0, nch), b in zip(calls, bases):
        rel = slot_sg[:, :, c0:c0 + nch, :] - b
        flat = rel.reshape(NCORES, NB, nch * 128)
        for ci in range(NCORES):
            for bi in range(NB):
                idx16[ci, bi, :, c0 * 8:(c0 + nch) * 8] = wrap_idx_flat(flat[ci, bi])

    dstrel = np.zeros((NCORES, NB, NCH + 1, 128), np.float32)
    selfvalid = (np.arange(cfg.NPAD) < cfg.NLOC).reshape(NB, 128)
    dstrel[:, :, 0, :] = np.where(selfvalid[None], np.arange(128)[None, None], -1.0)
    dstrel[:, :, 1:, :] = slot_dr

    return dict(idx16=idx16, dstrel=dstrel, bases=bases, calls=calls,
                slot_oid=slot_oid)


def prep_inputs(cfg, inputs):
    x = np.asarray(inputs["x"], np.float32)
    g = prep_graph(cfg, np.asarray(inputs["edge_index"], np.int64))

    iota_row = np.tile(np.arange(128, dtype=np.float32), (128, 1))
    ident = np.eye(128, dtype=np.float32)

    def bcast(v):
        return np.tile(np.asarray(v, np.float32).reshape(1, -1), (128, 1))

    shared = dict(
        enc_W=np.ascontiguousarray(inputs["enc_W"], np.float32),
        Wl0=np.ascontiguousarray(inputs["Wl0"], np.float32),
        Wr0=np.ascontiguousarray(inputs["Wr0"], np.float32),
        Wl1=np.ascontiguousarray(inputs["Wl1"], np.float32),
        Wr1=np.ascontiguousarray(inputs["Wr1"], np.float32),
        bl0_b=bcast(np.asarray(inputs["enc_b"], np.float32)
                    @ np.asarray(inputs["Wl0"], np.float32)
                    + np.asarray(inputs["bl0"], np.float32)),
        br0_b=bcast(np.asarray(inputs["enc_b"], np.float32)
                    @ np.asarray(inputs["Wr0"], np.float32)
                    + np.asarray(inputs["br0"], np.float32)),
        bl1_b=bcast(inputs["bl1"]), br1_b=bcast(inputs["br1"]),
        bias0_b=bcast(np.asarray(inputs["bias0"], np.float32)
                      + np.asarray(inputs["enc_b"], np.float32)),
        bias1_b=bcast(inputs["bias1"]),
        att0_b=bcast(np.asarray(inputs["att0"], np.float32).reshape(-1)),
        att1_b=bcast(np.asarray(inputs["att1"], np.float32).reshape(-1)),
        clfw_b=bcast(np.asarray(inputs["clf_W"], np.float32).reshape(-1)),
        iota_row=iota_row, ident=ident,
    )
    clf_b = float(np.asarray(inputs["clf_b"]).reshape(-1)[0])

    in_maps = []
    for c in range(NCORES):
        xc = x[c * cfg.NLOC:(c + 1) * cfg.NLOC]
        xT = np.zeros((cfg.IN_DIM, cfg.NPAD), np.float32)
        xT[:, :cfg.NLOC] = xc.T
        m = dict(shared)
        m["xT"] = xT
        m["idx16"] = g["idx16"][c]
        m["dstrel"] = g["dstrel"][c]
        in_maps.append(m)

    meta = dict(bases=g["bases"], calls=g["calls"], slot_oid=g["slot_oid"],
                clf_b=clf_b)
    return in_maps, meta


# ---------------------------------------------------------------- builder

def build_program(cfg, bases):
    HID_, H_, D_ = cfg.HID, cfg.H, cfg.D
    NB, NCH, NPAD, NGPAD = cfg.NB, cfg.NCH, cfg.NPAD, cfg.NGPAD
    KIN = cfg.KIN
    CALLS = []
    c0 = 0
    for nch in cfg.CALL_CHUNKS:
        CALLS.append((c0, nch))
        c0 += nch

    nc = bacc.Bacc("TRN2", target_bir_lowering=False, debug=False,
                   num_devices=NCORES)

    xT = nc.declare_dram_parameter("xT", [cfg.IN_DIM, NPAD], F32, isOutput=False)
    idx16 = nc.declare_dram_parameter("idx16", [NB, 128, NCH * 8], I16, isOutput=False)
    dstrel = nc.declare_dram_parameter("dstrel", [NB, NCH + 1, 128], F32, isOutput=False)
    enc_W = nc.declare_dram_parameter("enc_W", [cfg.IN_DIM, HID_], F32, isOutput=False)
    Ws = {}
    for nm in ("Wl0", "Wr0", "Wl1", "Wr1"):
        Ws[nm] = nc.declare_dram_parameter(nm, [HID_, HID_], F32, isOutput=False)
    bc = {}
    for nm in ("bl0_b", "br0_b", "bl1_b", "br1_b", "bias0_b",
               "bias1_b", "att0_b", "att1_b", "clfw_b"):
        bc[nm] = nc.declare_dram_parameter(nm, [128, HID_], F32, isOutput=False)
    iota_d = nc.declare_dram_parameter("iota_row", [128, 128], F32, isOutput=False)
    ident_d = nc.declare_dram_parameter("ident", [128, 128], F32, isOutput=False)

    preds_o = nc.declare_dram_parameter("preds", [NPAD, 1], F32, isOutput=True)
    alpha_o = [
        nc.declare_dram_parameter(f"alpha{l}", [NB, NCH + 1, 128, H_], F32,
                                  isOutput=True)
        for l in range(2)
    ]

    def dram(name, shape):
        return nc.dram_tensor(name, shape, F32)

    h_nm = [dram("h0_nm", [NPAD, HID_]), dram("h1_nm", [NPAD, HID_])]
    xl_sl = [dram("xl0_sl", [NPAD, HID_]), dram("xl1_sl", [NPAD, HID_])]
    xr_sl = [dram("xr0_sl", [NPAD, HID_]), dram("xr1_sl", [NPAD, HID_])]
    xl_full = [
        nc.dram_tensor("xl0_full", [NGPAD, HID_], F32, addr_space="Shared"),
        nc.dram_tensor("xl1_full", [NGPAD, HID_], F32, addr_space="Shared"),
    ]

    with tile.TileContext(nc) as tc, nc.allow_low_precision(
            reason="f32r tiles are bit-identical to f32"):
        with tc.tile_pool(name="const", bufs=1) as constp:
            encW_sb = constp.tile([128, KIN, HID_], F32R)
            nc.sync.dma_start(
                encW_sb[:],
                enc_W[:].rearrange("(k p) f -> p k f", p=128).bitcast(F32R))
            W_sb = {}
            for nm in Ws:
                W_sb[nm] = constp.tile([128, 2, HID_], F32R, tag=f"W{nm}",
                                       name=f"W{nm}sb")
                nc.sync.dma_start(
                    W_sb[nm][:],
                    Ws[nm][:].rearrange("(k p) f -> p k f", p=128).bitcast(F32R))
            bc_sb = {}
            for nm in bc:
                bc_sb[nm] = constp.tile([128, HID_], F32, tag=f"b{nm}",
                                        name=f"bc{nm}sb")
                nc.sync.dma_start(bc_sb[nm][:], bc[nm][:])
            iota_sb = constp.tile([128, 128], F32)
            nc.sync.dma_start(iota_sb[:], iota_d[:])
            id_sb = constp.tile([128, 128], F32R)
            nc.sync.dma_start(id_sb[:], ident_d[:].bitcast(F32R))

            # ---- Phase A: encoder + layer-0 transforms
            with (
                tc.tile_pool(name="enc_in", bufs=3) as enc_in,
                tc.tile_pool(name="enc_ps", bufs=2, space="PSUM") as enc_ps,
                tc.tile_pool(name="enc_out", bufs=3) as enc_out,
            ):
                nchunks = NPAD // 512 + (1 if NPAD % 512 else 0)
                for ch in range(nchunks):
                    n0 = ch * 512
                    nn = min(512, NPAD - n0)
                    xt_t = enc_in.tile([128, KIN, nn], F32R, tag="xt")
                    nc.sync.dma_start(
                        xt_t[:],
                        xT[:, n0:n0 + nn].rearrange(
                            "(k p) n -> p k n", p=128).bitcast(F32R))
                    h0T_sb = enc_out.tile([128, 2, nn], F32R, tag="h0T")
                    for f in range(2):
                        ps = enc_ps.tile([128, 512], F32, tag="hps")
                        for k in range(KIN):
                            nc.tensor.matmul(
                                ps[:, :nn], encW_sb[:, k, f * 128:(f + 1) * 128],
                                xt_t[:, k, :], start=(k == 0), stop=(k == KIN - 1))
                        nc.scalar.copy(h0T_sb[:, f, :], ps[:, :nn])
                    for sb in range(nn // 128):
                        nsub = n0 + sb * 128
                        h0_t = enc_out.tile([128, HID_], F32, tag="h0nm")
                        for f in range(2):
                            tp = enc_ps.tile([128, 128], F32R, tag="tp")
                            nc.tensor.transpose(
                                tp[:], h0T_sb[:, f, sb * 128:(sb + 1) * 128],
                                id_sb[:])
                            nc.scalar.copy(h0_t[:, f * 128:(f + 1) * 128], tp[:])
                        nc.sync.dma_start(h_nm[0][nsub:nsub + 128, :], h0_t[:])
                        for nm, dstd, bias in (("Wl0", xl_sl[0], "bl0_b"),
                                               ("Wr0", xr_sl[0], "br0_b")):
                            ps = enc_ps.tile([128, HID_], F32, tag="xps")
                            for k in range(2):
                                nc.tensor.matmul(
                                    ps[:], h0T_sb[:, k, sb * 128:(sb + 1) * 128],
                                    W_sb[nm][:, k, :], start=(k == 0),
                                    stop=(k == 1))
                            xo = enc_out.tile([128, HID_], F32, tag="xo")
                            nc.vector.tensor_add(xo[:], ps[:], bc_sb[bias][:])
                            nc.sync.dma_start(dstd[nsub:nsub + 128, :], xo[:])

            nc.gpsimd.collective_compute(
                "AllGather", ALU.bypass, ins=[xl_sl[0][:]], outs=[xl_full[0][:]],
                replica_groups=[list(range(NCORES))])

            def edge_layer(l):
                att_nm = f"att{l}_b"
                bias_nm = f"bias{l}_b"
                with (
                    tc.tile_pool(name=f"gin{l}", bufs=2) as gin,
                    tc.tile_pool(name=f"oh{l}", bufs=2) as ohp,
                    tc.tile_pool(name=f"wk{l}", bufs=2) as wk,
                    tc.tile_pool(name=f"bl{l}", bufs=3) as blp,
                    tc.tile_pool(name=f"ps{l}", bufs=2, space="PSUM") as psp,
                    tc.tile_pool(name=f"pst{l}", bufs=1, space="PSUM") as pst,
                    tc.tile_pool(name=f"accd{l}", bufs=1, space="PSUM") as accd,
                    tc.tile_pool(name=f"acc{l}", bufs=2, space="PSUM") as accp,
                    tc.tile_pool(name=f"psr{l}", bufs=1, space="PSUM") as psrg,
                    tc.tile_pool(name=f"pse{l}", bufs=1, space="PSUM") as psep,
                ):
                    for b in range(NB):
                        r0 = b * 128
                        idx_sb = blp.tile([128, NCH * 8], I16, tag="idx")
                        nc.sync.dma_start(idx_sb[:], idx16[b])
                        dst_sb = blp.tile([128, NCH + 1], F32, tag="dst")
                        nc.sync.dma_start(
                            dst_sb[:], dstrel[b].rearrange("c p -> p c"))
                        xr_blk = blp.tile([128, HID_], F32R, tag="xr")
                        nc.sync.dma_start(
                            xr_blk[:], xr_sl[l][r0:r0 + 128, :].bitcast(F32R))
                        hprev = blp.tile([128, HID_], F32, tag="hp")
                        nc.sync.dma_start(hprev[:], h_nm[l][r0:r0 + 128, :])
                        xl_self = gin.tile([128, HID_], F32R, tag="xls")
                        nc.sync.dma_start(
                            xl_self[:], xl_sl[l][r0:r0 + 128, :].bitcast(F32R))

                        denom_ps = accd.tile([128, H_], F32, tag="denom")
                        out_ps = accp.tile([128, HID_], F32, tag="out")

                        # gather calls
                        xg = []
                        for gi, (c0, nch) in enumerate(CALLS):
                            t = gin.tile([128, 8, HID_], F32R, tag=f"g{gi}",
                                         name=f"g{gi}t")
                            base = bases[gi]
                            wrows = min(32768, NGPAD - base)
                            nc.gpsimd.dma_gather(
                                t[:, :nch, :],
                                xl_full[l][base:base + wrows, :].bitcast(F32R),
                                idx_sb[:, c0 * 8:(c0 + nch) * 8],
                                nch * 128, nch * 128, HID_)
                            xg.append(t)

                        # groups: (chunk ids, xl aps). group 0 = self loops.
                        groups = [([0], [xl_self[:]])]
                        for gi, (c0, nch) in enumerate(CALLS):
                            groups.append((
                                [c0 + 1 + j for j in range(nch)],
                                [xg[gi][:, j, :] for j in range(nch)],
                            ))

                        # ---------- pass 1 ----------
                        info = {}       # c -> (xl_ap, oht, oh)
                        pgroups = []    # per group: p tile AP [128, nch, H]
                        for gidx, (cids, xls) in enumerate(groups):
                            ng = len(cids)
                            lrc = wk.tile([128, 8, HID_], F32, tag="lrc",
                                          name="lrct")
                            for j, (c, xla) in enumerate(zip(cids, xls)):
                                if c == 0:
                                    oht = oh = None
                                else:
                                    oht = ohp.tile([128, 128], F32R,
                                                   tag=f"oht{c}", name=f"oht{c}t")
                                    nc.vector.tensor_scalar(
                                        oht[:], iota_sb[:],
                                        dst_sb[:, c:c + 1], None, ALU.is_equal)
                                    tp = pst.tile([128, 128], F32R, tag="ohtp")
                                    nc.tensor.transpose(tp[:], oht[:], id_sb[:])
                                    oh = ohp.tile([128, 128], F32R,
                                                  tag=f"oh{c}", name=f"oh{c}t")
                                    nc.scalar.copy(oh[:], tp[:])
                                info[c] = (xla, oht, oh)
                                ef = psp.tile([128, HID_], F32, tag="ef")
                                nc.tensor.matmul(
                                    ef[:], oh[:] if oh is not None else id_sb[:],
                                    xr_blk[:], start=True, stop=False)
                                nc.tensor.matmul(ef[:], id_sb[:], xla,
                                                 start=False, stop=True)
                                nc.scalar.activation(lrc[:, j, :], ef[:],
                                                     AF.Prelu, alpha=cfg.NEG)
                            # batched logits for the group
                            attb = bc_sb[att_nm][:].rearrange(
                                "p (a f) -> p a f", a=1).broadcast_to(
                                [128, ng, HID_])
                            nc.vector.tensor_tensor(
                                lrc[:, :ng, :], lrc[:, :ng, :], attb, ALU.mult)
                            lgc = wk.tile([128, 8, H_], F32, tag="lgc",
                                          name="lgct")
                            nc.vector.tensor_reduce(
                                lgc[:, :ng, :],
                                lrc[:, :ng, :].rearrange(
                                    "p a (h d) -> p a h d", h=H_),
                                mybir.AxisListType.X, ALU.add)
                            pc = ohp.tile([128, 8, H_], F32R, tag=f"pg{gidx}",
                                          name=f"pg{gidx}t")
                            nc.scalar.activation(pc[:, :ng, :], lgc[:, :ng, :],
                                                 AF.Exp)
                            pgroups.append(pc)
                            for j, c in enumerate(cids):
                                _, oht, _ = info[c]
                                nc.tensor.matmul(
                                    denom_ps[:], oht[:] if oht is not None
                                    else id_sb[:], pc[:, j, :],
                                    start=(c == 0), stop=(c == NCH))

                        dsum = wk.tile([128, H_], F32, tag="ds")
                        nc.vector.tensor_scalar_add(dsum[:], denom_ps[:], 1e-16)
                        recip = blp.tile([128, H_], F32R, tag="rc")
                        nc.vector.reciprocal(recip[:], dsum[:])

                        # ---------- pass 2 ----------
                        for gidx, (cids, xls) in enumerate(groups):
                            ng = len(cids)
                            pc = pgroups[gidx]
                            rgc = psrg.tile([128, 8 * H_], F32, tag="rg")
                            for j, c in enumerate(cids):
                                _, oht, oh = info[c]
                                nc.tensor.matmul(
                                    rgc[:, j * H_:(j + 1) * H_],
                                    oh[:] if oh is not None else id_sb[:],
                                    recip[:], start=True, stop=True)
                            alc = wk.tile([128, 8, H_], F32, tag="alc",
                                          name="alct")
                            nc.vector.tensor_tensor(
                                alc[:, :ng, :], pc[:, :ng, :],
                                rgc[:, :ng * H_].rearrange(
                                    "p (a h) -> p a h", h=H_), ALU.mult)
                            c0g = cids[0]
                            nc.sync.dma_start(
                                alpha_o[l][b, c0g:c0g + ng].rearrange(
                                    "c p h -> p c h"), alc[:, :ng, :])
                            scc = wk.tile([128, 8, HID_], F32R, tag="lrc",
                                          name="scct")
                            for j, (c, xla) in enumerate(zip(cids, xls)):
                                pass
                            xg_in = (xl_self[:].rearrange(
                                "p (a f) -> p a f", a=1) if gidx == 0
                                else xg[gidx - 1][:, :ng, :])
                            nc.vector.tensor_tensor(
                                scc[:, :ng, :].rearrange(
                                    "p a (h d) -> p a h d", h=H_),
                                xg_in.rearrange("p a (h d) -> p a h d", h=H_),
                                alc[:, :ng, :].rearrange(
                                    "p a (h o) -> p a h o", o=1).broadcast_to(
                                    [128, ng, H_, D_]),
                                ALU.mult)
                            for j, c in enumerate(cids):
                                _, oht, _ = info[c]
                                nc.tensor.matmul(
                                    out_ps[:], oht[:] if oht is not None
                                    else id_sb[:], scc[:, j, :],
                                    start=(c == 0), stop=(c == NCH))

                        t1 = wk.tile([128, HID_], F32, tag="t1")
                        nc.vector.tensor_add(t1[:], out_ps[:], bc_sb[bias_nm][:])
                        t2 = wk.tile([128, HID_], F32, tag="t2")
                        nc.vector.tensor_add(t2[:], t1[:], hprev[:])
                        mn = wk.tile([128, HID_], F32, tag="mn")
                        nc.vector.tensor_scalar_min(mn[:], t2[:], 0.0)
                        ex = wk.tile([128, HID_], F32, tag="ex")
                        nc.scalar.activation(ex[:], mn[:], AF.Exp)
                        mx = wk.tile([128, HID_], F32, tag="mx")
                        nc.vector.tensor_scalar(mx[:], t2[:], 0.0, -1.0,
                                                ALU.max, ALU.add)
                        hout = blp.tile([128, HID_], F32R, tag="ho")
                        nc.vector.tensor_add(hout[:], ex[:], mx[:])
                        if l == 0:
                            nc.sync.dma_start(
                                h_nm[1][r0:r0 + 128, :].bitcast(F32R), hout[:])
                            h1T_t = blp.tile([128, 2, 128], F32R, tag="h1T")
                            for f in range(2):
                                tp2 = psep.tile([128, HID_], F32R, tag="ep",
                                                name="eptp")
                                nc.tensor.transpose(
                                    tp2[:, :128],
                                    hout[:, f * 128:(f + 1) * 128], id_sb[:])
                                nc.scalar.copy(h1T_t[:, f, :], tp2[:, :128])
                            for nm, dstd, bias in (("Wl1", xl_sl[1], "bl1_b"),
                                                   ("Wr1", xr_sl[1], "br1_b")):
                                ps = psep.tile([128, HID_], F32, tag="ep",
                                               name="epps")
                                for k in range(2):
                                    nc.tensor.matmul(
                                        ps[:], h1T_t[:, k, :], W_sb[nm][:, k, :],
                                        start=(k == 0), stop=(k == 1))
                                xo = wk.tile([128, HID_], F32, tag="xo1")
                                nc.vector.tensor_add(xo[:], ps[:], bc_sb[bias][:])
                                nc.sync.dma_start(dstd[r0:r0 + 128, :], xo[:])
                        else:
                            t3 = wk.tile([128, HID_], F32, tag="t3")
                            nc.vector.tensor_mul(t3[:], hout[:],
                                                 bc_sb["clfw_b"][:])
                            pr = wk.tile([128, 1], F32, tag="pr")
                            nc.vector.tensor_reduce(
                                pr[:], t3[:], mybir.AxisListType.X, ALU.add)
                            nc.sync.dma_start(preds_o[r0:r0 + 128, :], pr[:])

            edge_layer(0)
            nc.gpsimd.collective_compute(
                "AllGather", ALU.bypass, ins=[xl_sl[1][:]], outs=[xl_full[1][:]],
                replica_groups=[list(range(NCORES))])
            edge_layer(1)

    nc.compile()
    return nc


# ---------------------------------------------------------------- unshard

def unshard(cfg, results, meta):
    Etot = cfg.E + cfg.N
    preds = np.concatenate(
        [results[c]["preds"][:cfg.NLOC, 0] for c in range(NCORES)])
    preds = (preds + np.float32(meta["clf_b"])).astype(np.float32)

    alphas = []
    for l in range(2):
        af = np.zeros((Etot, cfg.H), np.float32)
        for c in range(NCORES):
            a = results[c][f"alpha{l}"]
            nodes = np.arange(cfg.NPAD)
            valid = nodes < cfg.NLOC
            sl = a[:, 0, :, :].reshape(cfg.NPAD, cfg.H)
            af[cfg.E + c * cfg.NLOC + nodes[valid]] = sl[valid]
            oid = meta["slot_oid"][c].reshape(-1)
            rnd = a[:, 1:, :, :].reshape(-1, cfg.H)
            m = oid >= 0
            af[oid[m]] = rnd[m]
        alphas.append(af)
    return preds, alphas[0], alphas[1]


# ---------------------------------------------------------------- entry

_CACHE = {}


def _run(inputs, trace=False):
    edge_index = np.asarray(inputs["edge_index"], np.int64)
    last_err = None
    for nch in (18, 20, 22, 26, 32):
        for call_size in (8, 4, 2):
            cfg = make_cfg(N, E, IN_DIM, nch, call_size)
            try:
                in_maps, meta = prep_inputs(cfg, inputs)
            except ValueError as e:
                last_err = e
                continue
            key = (nch, cfg.CALL_CHUNKS, tuple(meta["bases"]))
            if key not in _CACHE:
                _CACHE[key] = build_program(cfg, meta["bases"])
            nc = _CACHE[key]
            res = run_bass_kernel_spmd(nc, in_maps, list(range(NCORES)),
                                       trace=trace)
            return cfg, meta, res
    raise RuntimeError(f"no feasible cfg found: {last_err}")


def kernel(**inputs):
    cfg, meta, res = _run(inputs)
    return unshard(cfg, res.results, meta)


# revision 4
# speedup vs baseline: 1.0252x; 1.0252x over previous
"""nn_GATv2Net kernel for 8 TRN2 NeuronCores (self-contained).

kernel(**inputs) takes the FULL unsharded inputs of the reference
(x [50000,1280], edge_index [2,800000] plus weights) and returns
(preds [50000], alpha0 [850000,4], alpha1 [850000,4]) as float32, matching
reference.reference().

Distribution: nodes block-partitioned across the 8 cores; each edge is owned
by the core holding its dst node. Edges are grouped into per-core blocks of
128 consecutive local dst nodes and sorted by src within the block. Per-edge
src features are fetched with SWDGE dma_gather from an AllGather-replicated
xl table; segment softmax + aggregation run as one-hot f32r matmuls in PSUM.
"""
import dataclasses
import numpy as np

import concourse.bacc as bacc
import concourse.mybir as mybir
from concourse import tile
from concourse.bass_utils import run_bass_kernel_spmd

F32 = mybir.dt.float32
F32R = mybir.dt.float32r
I16 = mybir.dt.int16
AF = mybir.ActivationFunctionType
ALU = mybir.AluOpType

NCORES = 8
N, E, IN_DIM, HID, H, D = 50000, 800000, 1280, 256, 4, 64
NEG = 0.2


@dataclasses.dataclass
class Cfg:
    N: int = N
    E: int = E
    IN_DIM: int = IN_DIM
    HID: int = HID
    H: int = H
    D: int = D
    NEG: float = NEG
    NLOC: int = N // NCORES
    NB: int = (N // NCORES + 127) // 128
    NPAD: int = ((N // NCORES + 127) // 128) * 128
    NCH: int = 18
    CALL_CHUNKS: tuple = (8, 8, 2)

    @property
    def NGPAD(self):
        return NCORES * self.NPAD

    @property
    def KIN(self):
        return self.IN_DIM // 128


def make_cfg(N_, E_, IN_DIM_, nch, call_size=8):
    nloc = N_ // NCORES
    nb = (nloc + 127) // 128
    cc = []
    left = nch
    while left > 0:
        cc.append(min(call_size, left))
        left -= cc[-1]
    return Cfg(N=N_, E=E_, IN_DIM=IN_DIM_, NLOC=nloc, NB=nb, NPAD=nb * 128,
               NCH=nch, CALL_CHUNKS=tuple(cc))


# ---------------------------------------------------------------- host prep

def wrap_idx_flat(idx):
    """[n] int (n % 16 == 0) -> [128, n//16] int16 SWDGE wrapped layout."""
    n = idx.shape[0]
    a = idx.astype(np.int16).reshape(n // 16, 16)
    return np.tile(a.T, (8, 1))


def prep_graph(cfg, edge_index):
    src = np.asarray(edge_index[0], np.int64)
    dst = np.asarray(edge_index[1], np.int64)

    core = dst // cfg.NLOC
    ldst = dst - core * cfg.NLOC
    blk = ldst // 128
    drel = ldst % 128
    sg = (src // cfg.NLOC) * cfg.NPAD + (src % cfg.NLOC)

    order = np.lexsort((sg, blk, core))
    core_s, blk_s, drel_s, sg_s = core[order], blk[order], drel[order], sg[order]

    NCH, NB = cfg.NCH, cfg.NB
    cap = NCH * 128
    key = core_s * NB + blk_s
    bounds = np.searchsorted(key, np.arange(NCORES * NB + 1))
    counts = np.diff(bounds)
    if counts.max() > cap:
        raise ValueError(f"block overflow: {counts.max()} > {cap}")

    slot_sg = np.zeros((NCORES * NB, cap), np.int64)
    slot_dr = np.full((NCORES * NB, cap), -1.0, np.float32)
    slot_oid = np.full((NCORES * NB, cap), -1, np.int64)
    ar = np.arange(cap)
    mask = ar[None, :] < counts[:, None]
    idx_flat = np.nonzero(mask)
    slot_sg[idx_flat] = sg_s
    slot_dr[idx_flat] = drel_s.astype(np.float32)
    slot_oid[idx_flat] = order
    lastv = slot_sg[np.arange(NCORES * NB), np.maximum(counts - 1, 0)]
    slot_sg[~mask] = np.repeat(lastv, cap - counts)
    slot_sg[counts == 0] = 0

    slot_sg = slot_sg.reshape(NCORES, NB, NCH, 128)
    slot_dr = slot_dr.reshape(NCORES, NB, NCH, 128)
    slot_oid = slot_oid.reshape(NCORES, NB, NCH, 128)

    bases, calls = [], []
    c0 = 0
    for nch in cfg.CALL_CHUNKS:
        seg = slot_sg[:, :, c0:c0 + nch, :]
        b = int(seg.min())
        span = int(seg.max()) - b
        if span > 32767:
            raise ValueError(f"span {span} > 32767 (chunks {c0}..{c0+nch})")
        bases.append(b)
        calls.append((c0, nch))
        c0 += nch

    idx16 = np.zeros((NCORES, NB, 128, NCH * 8), np.int16)
    for (c0, nch), b in zip(calls, bases):
        rel = slot_sg[:, :, c0:c0 + nch, :] - b
        flat = rel.reshape(NCORES, NB, nch * 128)
        for ci in range(NCORES):
            for bi in range(NB):
                idx16[ci, bi, :, c0 * 8:(c0 + nch) * 8] = wrap_idx_flat(flat[ci, bi])

    dstrel = np.zeros((NCORES, NB, NCH + 1, 128), np.float32)
    selfvalid = (np.arange(cfg.NPAD) < cfg.NLOC).reshape(NB, 128)
    dstrel[:, :, 0, :] = np.where(selfvalid[None], np.arange(128)[None, None], -1.0)
    dstrel[:, :, 1:, :] = slot_dr

    return dict(idx16=idx16, dstrel=dstrel, bases=bases, calls=calls,
                slot_oid=slot_oid)


def prep_inputs(cfg, inputs):
    x = np.asarray(inputs["x"], np.float32)
    g = prep_graph(cfg, np.asarray(inputs["edge_index"], np.int64))

    iota_row = np.tile(np.arange(128, dtype=np.float32), (128, 1))
    ident = np.eye(128, dtype=np.float32)

    def bcast(v):
        return np.tile(np.asarray(v, np.float32).reshape(1, -1), (128, 1))

    shared = dict(
        enc_W=np.ascontiguousarray(inputs["enc_W"], np.float32),
        Wl0=np.ascontiguousarray(inputs["Wl0"], np.float32),
        Wr0=np.ascontiguousarray(inputs["Wr0"], np.float32),
        Wl1=np.ascontiguousarray(inputs["Wl1"], np.float32),
        Wr1=np.ascontiguousarray(inputs["Wr1"], np.float32),
        bl0_b=bcast(np.asarray(inputs["enc_b"], np.float32)
                    @ np.asarray(inputs["Wl0"], np.float32)
                    + np.asarray(inputs["bl0"], np.float32)),
        br0_b=bcast(np.asarray(inputs["enc_b"], np.float32)
                    @ np.asarray(inputs["Wr0"], np.float32)
                    + np.asarray(inputs["br0"], np.float32)),
        bl1_b=bcast(inputs["bl1"]), br1_b=bcast(inputs["br1"]),
        bias0_b=bcast(np.asarray(inputs["bias0"], np.float32)
                      + np.asarray(inputs["enc_b"], np.float32)),
        bias1_b=bcast(inputs["bias1"]),
        att0_b=bcast(np.asarray(inputs["att0"], np.float32).reshape(-1)),
        att1_b=bcast(np.asarray(inputs["att1"], np.float32).reshape(-1)),
        clfw_b=bcast(np.asarray(inputs["clf_W"], np.float32).reshape(-1)),
        iota_row=iota_row, ident=ident,
    )
    clf_b = float(np.asarray(inputs["clf_b"]).reshape(-1)[0])

    in_maps = []
    for c in range(NCORES):
        xc = x[c * cfg.NLOC:(c + 1) * cfg.NLOC]
        xT = np.zeros((cfg.IN_DIM, cfg.NPAD), np.float32)
        xT[:, :cfg.NLOC] = xc.T
        m = dict(shared)
        m["xT"] = xT
        m["idx16"] = g["idx16"][c]
        m["dstrel"] = g["dstrel"][c]
        in_maps.append(m)

    meta = dict(bases=g["bases"], calls=g["calls"], slot_oid=g["slot_oid"],
                clf_b=clf_b)
    return in_maps, meta


# ---------------------------------------------------------------- builder

def build_program(cfg, bases):
    HID_, H_, D_ = cfg.HID, cfg.H, cfg.D
    NB, NCH, NPAD, NGPAD = cfg.NB, cfg.NCH, cfg.NPAD, cfg.NGPAD
    KIN = cfg.KIN
    CALLS = []
    c0 = 0
    for nch in cfg.CALL_CHUNKS:
        CALLS.append((c0, nch))
        c0 += nch

    nc = bacc.Bacc("TRN2", target_bir_lowering=False, debug=False,
                   num_devices=NCORES)

    xT = nc.declare_dram_parameter("xT", [cfg.IN_DIM, NPAD], F32, isOutput=False)
    idx16 = nc.declare_dram_parameter("idx16", [NB, 128, NCH * 8], I16, isOutput=False)
    dstrel = nc.declare_dram_parameter("dstrel", [NB, NCH + 1, 128], F32, isOutput=False)
    enc_W = nc.declare_dram_parameter("enc_W", [cfg.IN_DIM, HID_], F32, isOutput=False)
    Ws = {}
    for nm in ("Wl0", "Wr0", "Wl1", "Wr1"):
        Ws[nm] = nc.declare_dram_parameter(nm, [HID_, HID_], F32, isOutput=False)
    bc = {}
    for nm in ("bl0_b", "br0_b", "bl1_b", "br1_b", "bias0_b",
               "bias1_b", "att0_b", "att1_b", "clfw_b"):
        bc[nm] = nc.declare_dram_parameter(nm, [128, HID_], F32, isOutput=False)
    iota_d = nc.declare_dram_parameter("iota_row", [128, 128], F32, isOutput=False)
    ident_d = nc.declare_dram_parameter("ident", [128, 128], F32, isOutput=False)

    preds_o = nc.declare_dram_parameter("preds", [NPAD, 1], F32, isOutput=True)
    alpha_o = [
        nc.declare_dram_parameter(f"alpha{l}", [NB, NCH + 1, 128, H_], F32,
                                  isOutput=True)
        for l in range(2)
    ]

    def dram(name, shape):
        return nc.dram_tensor(name, shape, F32)

    h_nm = [dram("h0_nm", [NPAD, HID_]), dram("h1_nm", [NPAD, HID_])]
    xl_sl = [dram("xl0_sl", [NPAD, HID_]), dram("xl1_sl", [NPAD, HID_])]
    xr_sl = [dram("xr0_sl", [NPAD, HID_]), dram("xr1_sl", [NPAD, HID_])]
    xl_full = [
        nc.dram_tensor("xl0_full", [NGPAD, HID_], F32, addr_space="Shared"),
        nc.dram_tensor("xl1_full", [NGPAD, HID_], F32, addr_space="Shared"),
    ]

    with tile.TileContext(nc) as tc, nc.allow_low_precision(
            reason="f32r tiles are bit-identical to f32"):
        with tc.tile_pool(name="const", bufs=1) as constp:
            encW_sb = constp.tile([128, KIN, HID_], F32R)
            nc.sync.dma_start(
                encW_sb[:],
                enc_W[:].rearrange("(k p) f -> p k f", p=128).bitcast(F32R))
            W_sb = {}
            for nm in Ws:
                W_sb[nm] = constp.tile([128, 2, HID_], F32R, tag=f"W{nm}",
                                       name=f"W{nm}sb")
                nc.sync.dma_start(
                    W_sb[nm][:],
                    Ws[nm][:].rearrange("(k p) f -> p k f", p=128).bitcast(F32R))
            bc_sb = {}
            for nm in bc:
                bc_sb[nm] = constp.tile([128, HID_], F32, tag=f"b{nm}",
                                        name=f"bc{nm}sb")
                nc.sync.dma_start(bc_sb[nm][:], bc[nm][:])
            iota_sb = constp.tile([128, 128], F32)
            nc.sync.dma_start(iota_sb[:], iota_d[:])
            id_sb = constp.tile([128, 128], F32R)
            nc.sync.dma_start(id_sb[:], ident_d[:].bitcast(F32R))

            # ---- Phase A: encoder + layer-0 transforms
            with (
                tc.tile_pool(name="enc_in", bufs=3) as enc_in,
                tc.tile_pool(name="enc_ps", bufs=2, space="PSUM") as enc_ps,
                tc.tile_pool(name="enc_out", bufs=3) as enc_out,
            ):
                nchunks = NPAD // 512 + (1 if NPAD % 512 else 0)
                for ch in range(nchunks):
                    n0 = ch * 512
                    nn = min(512, NPAD - n0)
                    xt_t = enc_in.tile([128, KIN, nn], F32R, tag="xt")
                    nc.sync.dma_start(
                        xt_t[:],
                        xT[:, n0:n0 + nn].rearrange(
                            "(k p) n -> p k n", p=128).bitcast(F32R))
                    h0T_sb = enc_out.tile([128, 2, nn], F32R, tag="h0T")
                    for f in range(2):
                        ps = enc_ps.tile([128, 512], F32, tag="hps")
                        for k in range(KIN):
                            nc.tensor.matmul(
                                ps[:, :nn], encW_sb[:, k, f * 128:(f + 1) * 128],
                                xt_t[:, k, :], start=(k == 0), stop=(k == KIN - 1))
                        nc.scalar.copy(h0T_sb[:, f, :], ps[:, :nn])
                    for sb in range(nn // 128):
                        nsub = n0 + sb * 128
                        h0_t = enc_out.tile([128, HID_], F32, tag="h0nm")
                        for f in range(2):
                            tp = enc_ps.tile([128, 128], F32R, tag="tp")
                            nc.tensor.transpose(
                                tp[:], h0T_sb[:, f, sb * 128:(sb + 1) * 128],
                                id_sb[:])
                            nc.scalar.copy(h0_t[:, f * 128:(f + 1) * 128], tp[:])
                        nc.sync.dma_start(h_nm[0][nsub:nsub + 128, :], h0_t[:])
                        for nm, dstd, bias in (("Wl0", xl_sl[0], "bl0_b"),
                                               ("Wr0", xr_sl[0], "br0_b")):
                            ps = enc_ps.tile([128, HID_], F32, tag="xps")
                            for k in range(2):
                                nc.tensor.matmul(
                                    ps[:], h0T_sb[:, k, sb * 128:(sb + 1) * 128],
                                    W_sb[nm][:, k, :], start=(k == 0),
                                    stop=(k == 1))
                            xo = enc_out.tile([128, HID_], F32, tag="xo")
                            nc.vector.tensor_add(xo[:], ps[:], bc_sb[bias][:])
                            nc.sync.dma_start(dstd[nsub:nsub + 128, :], xo[:])

            nc.gpsimd.collective_compute(
                "AllGather", ALU.bypass, ins=[xl_sl[0][:]], outs=[xl_full[0][:]],
                replica_groups=[list(range(NCORES))])

            def edge_layer(l):
                att_nm = f"att{l}_b"
                bias_nm = f"bias{l}_b"
                with (
                    tc.tile_pool(name=f"gin{l}", bufs=2) as gin,
                    tc.tile_pool(name=f"oh{l}", bufs=2) as ohp,
                    tc.tile_pool(name=f"wk{l}", bufs=2) as wk,
                    tc.tile_pool(name=f"bl{l}", bufs=3) as blp,
                    tc.tile_pool(name=f"ps{l}", bufs=1, space="PSUM") as psp,
                    tc.tile_pool(name=f"pst{l}", bufs=1, space="PSUM") as pst,
                    tc.tile_pool(name=f"acc{l}", bufs=2, space="PSUM") as accp,
                    tc.tile_pool(name=f"psr{l}", bufs=1, space="PSUM") as psrg,
                    tc.tile_pool(name=f"pse{l}", bufs=1, space="PSUM") as psep,
                ):
                    for b in range(NB):
                        r0 = b * 128
                        idx_sb = blp.tile([128, NCH * 8], I16, tag="idx")
                        nc.sync.dma_start(idx_sb[:], idx16[b])
                        dst_sb = blp.tile([128, NCH + 1], F32, tag="dst")
                        nc.sync.dma_start(
                            dst_sb[:], dstrel[b].rearrange("c p -> p c"))
                        xr_blk = blp.tile([128, HID_], F32R, tag="xr")
                        nc.sync.dma_start(
                            xr_blk[:], xr_sl[l][r0:r0 + 128, :].bitcast(F32R))
                        hprev = blp.tile([128, HID_], F32, tag="hp")
                        nc.sync.dma_start(hprev[:], h_nm[l][r0:r0 + 128, :])
                        xl_self = gin.tile([128, HID_], F32R, tag="xls")
                        nc.sync.dma_start(
                            xl_self[:], xl_sl[l][r0:r0 + 128, :].bitcast(F32R))

                        denom_ps = accp.tile([128, H_], F32, tag="denom")
                        out_ps = accp.tile([128, HID_], F32, tag="out")

                        # gather calls
                        xg = []
                        for gi, (c0, nch) in enumerate(CALLS):
                            t = gin.tile([128, 8, HID_], F32R, tag=f"g{gi}",
                                         name=f"g{gi}t")
                            base = bases[gi]
                            wrows = min(32768, NGPAD - base)
                            nc.gpsimd.dma_gather(
                                t[:, :nch, :],
                                xl_full[l][base:base + wrows, :].bitcast(F32R),
                                idx_sb[:, c0 * 8:(c0 + nch) * 8],
                                nch * 128, nch * 128, HID_)
                            xg.append(t)

                        # groups: (chunk ids, xl aps). group 0 = self loops.
                        groups = [([0], [xl_self[:]])]
                        for gi, (c0, nch) in enumerate(CALLS):
                            groups.append((
                                [c0 + 1 + j for j in range(nch)],
                                [xg[gi][:, j, :] for j in range(nch)],
                            ))

                        # ---------- pass 1 ----------
                        info = {}       # c -> (xl_ap, oht, oh)
                        pgroups = []    # per group: p tile AP [128, nch, H]
                        for gidx, (cids, xls) in enumerate(groups):
                            ng = len(cids)
                            lrc = wk.tile([128, 8, HID_], F32, tag="lrc",
                                          name="lrct")
                            for j, (c, xla) in enumerate(zip(cids, xls)):
                                if c == 0:
                                    oht = oh = None
                                else:
                                    oht = ohp.tile([128, 128], F32R,
                                                   tag=f"oht{c}", name=f"oht{c}t")
                                    nc.vector.tensor_scalar(
                                        oht[:], iota_sb[:],
                                        dst_sb[:, c:c + 1], None, ALU.is_equal)
                                    tp = pst.tile([128, 128], F32R, tag="ohtp")
                                    nc.tensor.transpose(tp[:], oht[:], id_sb[:])
                                    oh = ohp.tile([128, 128], F32R,
                                                  tag=f"oh{c}", name=f"oh{c}t")
                                    nc.scalar.copy(oh[:], tp[:])
                                info[c] = (xla, oht, oh)
                                ef = psp.tile([128, HID_], F32, tag="ef")
                                nc.tensor.matmul(
                                    ef[:], oh[:] if oh is not None else id_sb[:],
                                    xr_blk[:], start=True, stop=False)
                                nc.tensor.matmul(ef[:], id_sb[:], xla,
                                                 start=False, stop=True)
                                nc.scalar.activation(lrc[:, j, :], ef[:],
                                                     AF.Prelu, alpha=cfg.NEG)
                            # batched logits for the group
                            tmpc = wk.tile([128, 8, HID_], F32, tag="tmpc",
                                           name="tmpct")
                            attb = bc_sb[att_nm][:].rearrange(
                                "p (a f) -> p a f", a=1).broadcast_to(
                                [128, ng, HID_])
                            nc.vector.tensor_tensor(
                                tmpc[:, :ng, :], lrc[:, :ng, :], attb, ALU.mult)
                            lgc = wk.tile([128, 8, H_], F32, tag="lgc",
                                          name="lgct")
                            nc.vector.tensor_reduce(
                                lgc[:, :ng, :],
                                tmpc[:, :ng, :].rearrange(
                                    "p a (h d) -> p a h d", h=H_),
                                mybir.AxisListType.X, ALU.add)
                            pc = ohp.tile([128, 8, H_], F32R, tag=f"pg{gidx}",
                                          name=f"pg{gidx}t")
                            nc.scalar.activation(pc[:, :ng, :], lgc[:, :ng, :],
                                                 AF.Exp)
                            pgroups.append(pc)
                            for j, c in enumerate(cids):
                                _, oht, _ = info[c]
                                nc.tensor.matmul(
                                    denom_ps[:], oht[:] if oht is not None
                                    else id_sb[:], pc[:, j, :],
                                    start=(c == 0), stop=(c == NCH))

                        dsum = wk.tile([128, H_], F32, tag="ds")
                        nc.vector.tensor_scalar_add(dsum[:], denom_ps[:], 1e-16)
                        recip = blp.tile([128, H_], F32R, tag="rc")
                        nc.vector.reciprocal(recip[:], dsum[:])

                        # ---------- pass 2 ----------
                        for gidx, (cids, xls) in enumerate(groups):
                            ng = len(cids)
                            pc = pgroups[gidx]
                            rgc = psrg.tile([128, 8 * H_], F32, tag="rg")
                            for j, c in enumerate(cids):
                                _, oht, oh = info[c]
                                nc.tensor.matmul(
                                    rgc[:, j * H_:(j + 1) * H_],
                                    oh[:] if oh is not None else id_sb[:],
                                    recip[:], start=True, stop=True)
                            alc = wk.tile([128, 8, H_], F32, tag="alc",
                                          name="alct")
                            nc.vector.tensor_tensor(
                                alc[:, :ng, :], pc[:, :ng, :],
                                rgc[:, :ng * H_].rearrange(
                                    "p (a h) -> p a h", h=H_), ALU.mult)
                            c0g = cids[0]
                            nc.sync.dma_start(
                                alpha_o[l][b, c0g:c0g + ng].rearrange(
                                    "c p h -> p c h"), alc[:, :ng, :])
                            scc = wk.tile([128, 8, HID_], F32R, tag="scc",
                                          name="scct")
                            for j, (c, xla) in enumerate(zip(cids, xls)):
                                pass
                            xg_in = (xl_self[:].rearrange(
                                "p (a f) -> p a f", a=1) if gidx == 0
                                else xg[gidx - 1][:, :ng, :])
                            nc.vector.tensor_tensor(
                                scc[:, :ng, :].rearrange(
                                    "p a (h d) -> p a h d", h=H_),
                                xg_in.rearrange("p a (h d) -> p a h d", h=H_),
                                alc[:, :ng, :].rearrange(
                                    "p a (h o) -> p a h o", o=1).broadcast_to(
                                    [128, ng, H_, D_]),
                                ALU.mult)
                            for j, c in enumerate(cids):
                                _, oht, _ = info[c]
                                nc.tensor.matmul(
                                    out_ps[:], oht[:] if oht is not None
                                    else id_sb[:], scc[:, j, :],
                                    start=(c == 0), stop=(c == NCH))

                        t1 = wk.tile([128, HID_], F32, tag="t1")
                        nc.vector.tensor_add(t1[:], out_ps[:], bc_sb[bias_nm][:])
                        t2 = wk.tile([128, HID_], F32, tag="t2")
                        nc.vector.tensor_add(t2[:], t1[:], hprev[:])
                        mn = wk.tile([128, HID_], F32, tag="mn")
                        nc.vector.tensor_scalar_min(mn[:], t2[:], 0.0)
                        ex = wk.tile([128, HID_], F32, tag="ex")
                        nc.scalar.activation(ex[:], mn[:], AF.Exp)
                        mx = wk.tile([128, HID_], F32, tag="mx")
                        nc.vector.tensor_scalar(mx[:], t2[:], 0.0, -1.0,
                                                ALU.max, ALU.add)
                        hout = blp.tile([128, HID_], F32R, tag="ho")
                        nc.vector.tensor_add(hout[:], ex[:], mx[:])
                        if l == 0:
                            nc.sync.dma_start(
                                h_nm[1][r0:r0 + 128, :].bitcast(F32R), hout[:])
                            h1T_t = blp.tile([128, 2, 128], F32R, tag="h1T")
                            for f in range(2):
                                tp2 = psep.tile([128, HID_], F32R, tag="ep",
                                                name="eptp")
                                nc.tensor.transpose(
                                    tp2[:, :128],
                                    hout[:, f * 128:(f + 1) * 128], id_sb[:])
                                nc.scalar.copy(h1T_t[:, f, :], tp2[:, :128])
                            for nm, dstd, bias in (("Wl1", xl_sl[1], "bl1_b"),
                                                   ("Wr1", xr_sl[1], "br1_b")):
                                ps = psep.tile([128, HID_], F32, tag="ep",
                                               name="epps")
                                for k in range(2):
                                    nc.tensor.matmul(
                                        ps[:], h1T_t[:, k, :], W_sb[nm][:, k, :],
                                        start=(k == 0), stop=(k == 1))
                                xo = wk.tile([128, HID_], F32, tag="xo1")
                                nc.vector.tensor_add(xo[:], ps[:], bc_sb[bias][:])
                                nc.sync.dma_start(dstd[r0:r0 + 128, :], xo[:])
                        else:
                            t3 = wk.tile([128, HID_], F32, tag="t3")
                            nc.vector.tensor_mul(t3[:], hout[:],
                                                 bc_sb["clfw_b"][:])
                            pr = wk.tile([128, 1], F32, tag="pr")
                            nc.vector.tensor_reduce(
                                pr[:], t3[:], mybir.AxisListType.X, ALU.add)
                            nc.sync.dma_start(preds_o[r0:r0 + 128, :], pr[:])

            edge_layer(0)
            nc.gpsimd.collective_compute(
                "AllGather", ALU.bypass, ins=[xl_sl[1][:]], outs=[xl_full[1][:]],
                replica_groups=[list(range(NCORES))])
            edge_layer(1)

    nc.compile()
    return nc


# ---------------------------------------------------------------- unshard

def unshard(cfg, results, meta):
    Etot = cfg.E + cfg.N
    preds = np.concatenate(
        [results[c]["preds"][:cfg.NLOC, 0] for c in range(NCORES)])
    preds = (preds + np.float32(meta["clf_b"])).astype(np.float32)

    alphas = []
    for l in range(2):
        af = np.zeros((Etot, cfg.H), np.float32)
        for c in range(NCORES):
            a = results[c][f"alpha{l}"]
            nodes = np.arange(cfg.NPAD)
            valid = nodes < cfg.NLOC
            sl = a[:, 0, :, :].reshape(cfg.NPAD, cfg.H)
            af[cfg.E + c * cfg.NLOC + nodes[valid]] = sl[valid]
            oid = meta["slot_oid"][c].reshape(-1)
            rnd = a[:, 1:, :, :].reshape(-1, cfg.H)
            m = oid >= 0
            af[oid[m]] = rnd[m]
        alphas.append(af)
    return preds, alphas[0], alphas[1]


# ---------------------------------------------------------------- entry

_CACHE = {}


def _run(inputs, trace=False):
    edge_index = np.asarray(inputs["edge_index"], np.int64)
    last_err = None
    for nch in (18, 20, 22, 26, 32):
        for call_size in (8, 4, 2):
            cfg = make_cfg(N, E, IN_DIM, nch, call_size)
            try:
                in_maps, meta = prep_inputs(cfg, inputs)
            except ValueError as e:
                last_err = e
                continue
            key = (nch, cfg.CALL_CHUNKS, tuple(meta["bases"]))
            if key not in _CACHE:
                _CACHE[key] = build_program(cfg, meta["bases"])
            nc = _CACHE[key]
            res = run_bass_kernel_spmd(nc, in_maps, list(range(NCORES)),
                                       trace=trace)
            return cfg, meta, res
    raise RuntimeError(f"no feasible cfg found: {last_err}")


def kernel(**inputs):
    cfg, meta, res = _run(inputs)
    return unshard(cfg, res.results, meta)


# revision 6
# speedup vs baseline: 1.0446x; 1.0189x over previous
"""nn_GATv2Net kernel for 8 TRN2 NeuronCores (self-contained).

kernel(**inputs) takes the FULL unsharded inputs of the reference
(x [50000,1280], edge_index [2,800000] plus weights) and returns
(preds [50000], alpha0 [850000,4], alpha1 [850000,4]) as float32, matching
reference.reference().

Distribution: nodes block-partitioned across the 8 cores; each edge is owned
by the core holding its dst node. Edges are grouped into per-core blocks of
128 consecutive local dst nodes and sorted by src within the block. Per-edge
src features are fetched with SWDGE dma_gather from an AllGather-replicated
xl table; segment softmax + aggregation run as one-hot f32r matmuls in PSUM.
"""
import dataclasses
import numpy as np

import concourse.bacc as bacc
import concourse.mybir as mybir
from concourse import tile
from concourse.bass_utils import run_bass_kernel_spmd

F32 = mybir.dt.float32
F32R = mybir.dt.float32r
I16 = mybir.dt.int16
AF = mybir.ActivationFunctionType
ALU = mybir.AluOpType

NCORES = 8
N, E, IN_DIM, HID, H, D = 50000, 800000, 1280, 256, 4, 64
NEG = 0.2


@dataclasses.dataclass
class Cfg:
    N: int = N
    E: int = E
    IN_DIM: int = IN_DIM
    HID: int = HID
    H: int = H
    D: int = D
    NEG: float = NEG
    NLOC: int = N // NCORES
    NB: int = (N // NCORES + 127) // 128
    NPAD: int = ((N // NCORES + 127) // 128) * 128
    NCH: int = 18
    CALL_CHUNKS: tuple = (8, 8, 2)

    @property
    def NGPAD(self):
        return NCORES * self.NPAD

    @property
    def KIN(self):
        return self.IN_DIM // 128


def make_cfg(N_, E_, IN_DIM_, nch, call_size=8):
    nloc = N_ // NCORES
    nb = (nloc + 127) // 128
    cc = []
    left = nch
    while left > 0:
        cc.append(min(call_size, left))
        left -= cc[-1]
    return Cfg(N=N_, E=E_, IN_DIM=IN_DIM_, NLOC=nloc, NB=nb, NPAD=nb * 128,
               NCH=nch, CALL_CHUNKS=tuple(cc))


# ---------------------------------------------------------------- host prep

def wrap_idx_flat(idx):
    """[n] int (n % 16 == 0) -> [128, n//16] int16 SWDGE wrapped layout."""
    n = idx.shape[0]
    a = idx.astype(np.int16).reshape(n // 16, 16)
    return np.tile(a.T, (8, 1))


def prep_graph(cfg, edge_index):
    src = np.asarray(edge_index[0], np.int64)
    dst = np.asarray(edge_index[1], np.int64)

    core = dst // cfg.NLOC
    ldst = dst - core * cfg.NLOC
    blk = ldst // 128
    drel = ldst % 128
    sg = (src // cfg.NLOC) * cfg.NPAD + (src % cfg.NLOC)

    order = np.lexsort((sg, blk, core))
    core_s, blk_s, drel_s, sg_s = core[order], blk[order], drel[order], sg[order]

    NCH, NB = cfg.NCH, cfg.NB
    cap = NCH * 128
    key = core_s * NB + blk_s
    bounds = np.searchsorted(key, np.arange(NCORES * NB + 1))
    counts = np.diff(bounds)
    if counts.max() > cap:
        raise ValueError(f"block overflow: {counts.max()} > {cap}")

    slot_sg = np.zeros((NCORES * NB, cap), np.int64)
    slot_dr = np.full((NCORES * NB, cap), -1.0, np.float32)
    slot_oid = np.full((NCORES * NB, cap), -1, np.int64)
    ar = np.arange(cap)
    mask = ar[None, :] < counts[:, None]
    idx_flat = np.nonzero(mask)
    slot_sg[idx_flat] = sg_s
    slot_dr[idx_flat] = drel_s.astype(np.float32)
    slot_oid[idx_flat] = order
    lastv = slot_sg[np.arange(NCORES * NB), np.maximum(counts - 1, 0)]
    slot_sg[~mask] = np.repeat(lastv, cap - counts)
    slot_sg[counts == 0] = 0

    slot_sg = slot_sg.reshape(NCORES, NB, NCH, 128)
    slot_dr = slot_dr.reshape(NCORES, NB, NCH, 128)
    slot_oid = slot_oid.reshape(NCORES, NB, NCH, 128)

    bases, calls = [], []
    c0 = 0
    for nch in cfg.CALL_CHUNKS:
        seg = slot_sg[:, :, c0:c0 + nch, :]
        b = int(seg.min())
        span = int(seg.max()) - b
        if span > 32767:
            raise ValueError(f"span {span} > 32767 (chunks {c0}..{c0+nch})")
        bases.append(b)
        calls.append((c0, nch))
        c0 += nch

    idx16 = np.zeros((NCORES, NB, 128, NCH * 8), np.int16)
    for (c0, nch), b in zip(calls, bases):
        rel = slot_sg[:, :, c0:c0 + nch, :] - b
        flat = rel.reshape(NCORES, NB, nch * 128)
        for ci in range(NCORES):
            for bi in range(NB):
                idx16[ci, bi, :, c0 * 8:(c0 + nch) * 8] = wrap_idx_flat(flat[ci, bi])

    dstrel = np.zeros((NCORES, NB, NCH + 1, 128), np.float32)
    selfvalid = (np.arange(cfg.NPAD) < cfg.NLOC).reshape(NB, 128)
    dstrel[:, :, 0, :] = np.where(selfvalid[None], np.arange(128)[None, None], -1.0)
    dstrel[:, :, 1:, :] = slot_dr

    return dict(idx16=idx16, dstrel=dstrel, bases=bases, calls=calls,
                slot_oid=slot_oid)


def prep_inputs(cfg, inputs):
    x = np.asarray(inputs["x"], np.float32)
    g = prep_graph(cfg, np.asarray(inputs["edge_index"], np.int64))

    iota_row = np.tile(np.arange(128, dtype=np.float32), (128, 1))
    ident = np.eye(128, dtype=np.float32)

    def bcast(v):
        return np.tile(np.asarray(v, np.float32).reshape(1, -1), (128, 1))

    shared = dict(
        enc_W=np.ascontiguousarray(inputs["enc_W"], np.float32),
        Wl0=np.ascontiguousarray(inputs["Wl0"], np.float32),
        Wr0=np.ascontiguousarray(inputs["Wr0"], np.float32),
        Wl1=np.ascontiguousarray(inputs["Wl1"], np.float32),
        Wr1=np.ascontiguousarray(inputs["Wr1"], np.float32),
        bl0_b=bcast(np.asarray(inputs["enc_b"], np.float32)
                    @ np.asarray(inputs["Wl0"], np.float32)
                    + np.asarray(inputs["bl0"], np.float32)),
        br0_b=bcast(np.asarray(inputs["enc_b"], np.float32)
                    @ np.asarray(inputs["Wr0"], np.float32)
                    + np.asarray(inputs["br0"], np.float32)),
        bl1_b=bcast(inputs["bl1"]), br1_b=bcast(inputs["br1"]),
        bias0_b=bcast(np.asarray(inputs["bias0"], np.float32)
                      + np.asarray(inputs["enc_b"], np.float32)),
        bias1_b=bcast(inputs["bias1"]),
        att0_b=bcast(np.asarray(inputs["att0"], np.float32).reshape(-1)),
        att1_b=bcast(np.asarray(inputs["att1"], np.float32).reshape(-1)),
        clfw_b=bcast(np.asarray(inputs["clf_W"], np.float32).reshape(-1)),
        iota_row=iota_row, ident=ident,
    )
    clf_b = float(np.asarray(inputs["clf_b"]).reshape(-1)[0])

    in_maps = []
    for c in range(NCORES):
        xc = x[c * cfg.NLOC:(c + 1) * cfg.NLOC]
        xT = np.zeros((cfg.IN_DIM, cfg.NPAD), np.float32)
        xT[:, :cfg.NLOC] = xc.T
        m = dict(shared)
        m["xT"] = xT
        m["idx16"] = g["idx16"][c]
        m["dstrel"] = g["dstrel"][c]
        in_maps.append(m)

    meta = dict(bases=g["bases"], calls=g["calls"], slot_oid=g["slot_oid"],
                clf_b=clf_b)
    return in_maps, meta


# ---------------------------------------------------------------- builder

def build_program(cfg, bases):
    HID_, H_, D_ = cfg.HID, cfg.H, cfg.D
    NB, NCH, NPAD, NGPAD = cfg.NB, cfg.NCH, cfg.NPAD, cfg.NGPAD
    KIN = cfg.KIN
    CALLS = []
    c0 = 0
    for nch in cfg.CALL_CHUNKS:
        CALLS.append((c0, nch))
        c0 += nch
    MAXG = max(cfg.CALL_CHUNKS)

    nc = bacc.Bacc("TRN2", target_bir_lowering=False, debug=False,
                   num_devices=NCORES)

    xT = nc.declare_dram_parameter("xT", [cfg.IN_DIM, NPAD], F32, isOutput=False)
    idx16 = nc.declare_dram_parameter("idx16", [NB, 128, NCH * 8], I16, isOutput=False)
    dstrel = nc.declare_dram_parameter("dstrel", [NB, NCH + 1, 128], F32, isOutput=False)
    enc_W = nc.declare_dram_parameter("enc_W", [cfg.IN_DIM, HID_], F32, isOutput=False)
    Ws = {}
    for nm in ("Wl0", "Wr0", "Wl1", "Wr1"):
        Ws[nm] = nc.declare_dram_parameter(nm, [HID_, HID_], F32, isOutput=False)
    bc = {}
    for nm in ("bl0_b", "br0_b", "bl1_b", "br1_b", "bias0_b",
               "bias1_b", "att0_b", "att1_b", "clfw_b"):
        bc[nm] = nc.declare_dram_parameter(nm, [128, HID_], F32, isOutput=False)
    iota_d = nc.declare_dram_parameter("iota_row", [128, 128], F32, isOutput=False)
    ident_d = nc.declare_dram_parameter("ident", [128, 128], F32, isOutput=False)

    preds_o = nc.declare_dram_parameter("preds", [NPAD, 1], F32, isOutput=True)
    alpha_o = [
        nc.declare_dram_parameter(f"alpha{l}", [NB, NCH + 1, 128, H_], F32,
                                  isOutput=True)
        for l in range(2)
    ]

    def dram(name, shape):
        return nc.dram_tensor(name, shape, F32)

    h_nm = [dram("h0_nm", [NPAD, HID_]), dram("h1_nm", [NPAD, HID_])]
    xl_sl = [dram("xl0_sl", [NPAD, HID_]), dram("xl1_sl", [NPAD, HID_])]
    xr_sl = [dram("xr0_sl", [NPAD, HID_]), dram("xr1_sl", [NPAD, HID_])]
    xl_full = [
        nc.dram_tensor("xl0_full", [NGPAD, HID_], F32, addr_space="Shared"),
        nc.dram_tensor("xl1_full", [NGPAD, HID_], F32, addr_space="Shared"),
    ]

    with tile.TileContext(nc) as tc, nc.allow_low_precision(
            reason="f32r tiles are bit-identical to f32"):
        with tc.tile_pool(name="const", bufs=1) as constp:
            encW_sb = constp.tile([128, KIN, HID_], F32R)
            nc.sync.dma_start(
                encW_sb[:],
                enc_W[:].rearrange("(k p) f -> p k f", p=128).bitcast(F32R))
            W_sb = {}
            for nm in Ws:
                W_sb[nm] = constp.tile([128, 2, HID_], F32R, tag=f"W{nm}",
                                       name=f"W{nm}sb")
                nc.sync.dma_start(
                    W_sb[nm][:],
                    Ws[nm][:].rearrange("(k p) f -> p k f", p=128).bitcast(F32R))
            bc_sb = {}
            for nm in bc:
                bc_sb[nm] = constp.tile([128, HID_], F32, tag=f"b{nm}",
                                        name=f"bc{nm}sb")
                nc.sync.dma_start(bc_sb[nm][:], bc[nm][:])
            iota_sb = constp.tile([128, 128], F32)
            nc.sync.dma_start(iota_sb[:], iota_d[:])
            id_sb = constp.tile([128, 128], F32R)
            nc.sync.dma_start(id_sb[:], ident_d[:].bitcast(F32R))

            # ---- Phase A: encoder + layer-0 transforms
            with (
                tc.tile_pool(name="enc_in", bufs=3) as enc_in,
                tc.tile_pool(name="enc_ps", bufs=2, space="PSUM") as enc_ps,
                tc.tile_pool(name="enc_out", bufs=3) as enc_out,
            ):
                nchunks = NPAD // 512 + (1 if NPAD % 512 else 0)
                for ch in range(nchunks):
                    n0 = ch * 512
                    nn = min(512, NPAD - n0)
                    xt_t = enc_in.tile([128, KIN, nn], F32R, tag="xt")
                    nc.sync.dma_start(
                        xt_t[:],
                        xT[:, n0:n0 + nn].rearrange(
                            "(k p) n -> p k n", p=128).bitcast(F32R))
                    h0T_sb = enc_out.tile([128, 2, nn], F32R, tag="h0T")
                    for f in range(2):
                        ps = enc_ps.tile([128, 512], F32, tag="hps")
                        for k in range(KIN):
                            nc.tensor.matmul(
                                ps[:, :nn], encW_sb[:, k, f * 128:(f + 1) * 128],
                                xt_t[:, k, :], start=(k == 0), stop=(k == KIN - 1))
                        nc.scalar.copy(h0T_sb[:, f, :], ps[:, :nn])
                    for sb in range(nn // 128):
                        nsub = n0 + sb * 128
                        h0_t = enc_out.tile([128, HID_], F32, tag="h0nm")
                        for f in range(2):
                            tp = enc_ps.tile([128, 128], F32R, tag="tp")
                            nc.tensor.transpose(
                                tp[:], h0T_sb[:, f, sb * 128:(sb + 1) * 128],
                                id_sb[:])
                            nc.scalar.copy(h0_t[:, f * 128:(f + 1) * 128], tp[:])
                        nc.sync.dma_start(h_nm[0][nsub:nsub + 128, :], h0_t[:])
                        for nm, dstd, bias in (("Wl0", xl_sl[0], "bl0_b"),
                                               ("Wr0", xr_sl[0], "br0_b")):
                            ps = enc_ps.tile([128, HID_], F32, tag="xps")
                            for k in range(2):
                                nc.tensor.matmul(
                                    ps[:], h0T_sb[:, k, sb * 128:(sb + 1) * 128],
                                    W_sb[nm][:, k, :], start=(k == 0),
                                    stop=(k == 1))
                            xo = enc_out.tile([128, HID_], F32, tag="xo")
                            nc.vector.tensor_add(xo[:], ps[:], bc_sb[bias][:])
                            nc.sync.dma_start(dstd[nsub:nsub + 128, :], xo[:])

            nc.gpsimd.collective_compute(
                "AllGather", ALU.bypass, ins=[xl_sl[0][:]], outs=[xl_full[0][:]],
                replica_groups=[list(range(NCORES))])

            def edge_layer(l):
                att_nm = f"att{l}_b"
                bias_nm = f"bias{l}_b"
                with (
                    tc.tile_pool(name=f"gin{l}", bufs=2) as gin,
                    tc.tile_pool(name=f"oh{l}", bufs=2) as ohp,
                    tc.tile_pool(name=f"wk{l}", bufs=2) as wk,
                    tc.tile_pool(name=f"bl{l}", bufs=3) as blp,
                    tc.tile_pool(name=f"ps{l}", bufs=1, space="PSUM") as psp,
                    tc.tile_pool(name=f"pst{l}", bufs=1, space="PSUM") as pst,
                    tc.tile_pool(name=f"acc{l}", bufs=2, space="PSUM") as accp,
                    tc.tile_pool(name=f"psr{l}", bufs=1, space="PSUM") as psrg,
                    tc.tile_pool(name=f"pse{l}", bufs=1, space="PSUM") as psep,
                ):
                    for b in range(NB):
                        r0 = b * 128
                        idx_sb = blp.tile([128, NCH * 8], I16, tag="idx")
                        nc.sync.dma_start(idx_sb[:], idx16[b])
                        dst_sb = blp.tile([128, NCH + 1], F32, tag="dst")
                        nc.sync.dma_start(
                            dst_sb[:], dstrel[b].rearrange("c p -> p c"))
                        xr_blk = blp.tile([128, HID_], F32R, tag="xr")
                        nc.sync.dma_start(
                            xr_blk[:], xr_sl[l][r0:r0 + 128, :].bitcast(F32R))
                        hprev = blp.tile([128, HID_], F32, tag="hp")
                        nc.sync.dma_start(hprev[:], h_nm[l][r0:r0 + 128, :])
                        xl_self = gin.tile([128, HID_], F32R, tag="xls")
                        nc.sync.dma_start(
                            xl_self[:], xl_sl[l][r0:r0 + 128, :].bitcast(F32R))

                        denom_ps = accp.tile([128, H_], F32, tag="denom")
                        out_ps = accp.tile([128, HID_], F32, tag="out")

                        # gather calls
                        xg = []
                        for gi, (c0, nch) in enumerate(CALLS):
                            t = gin.tile([128, MAXG, HID_], F32R, tag=f"g{gi}",
                                         name=f"g{gi}t")
                            base = bases[gi]
                            wrows = min(32768, NGPAD - base)
                            nc.gpsimd.dma_gather(
                                t[:, :nch, :],
                                xl_full[l][base:base + wrows, :].bitcast(F32R),
                                idx_sb[:, c0 * 8:(c0 + nch) * 8],
                                nch * 128, nch * 128, HID_)
                            xg.append(t)

                        # groups: (chunk ids, xl aps). group 0 = self loops.
                        groups = [([0], [xl_self[:]])]
                        for gi, (c0, nch) in enumerate(CALLS):
                            groups.append((
                                [c0 + 1 + j for j in range(nch)],
                                [xg[gi][:, j, :] for j in range(nch)],
                            ))

                        # ---------- pass 1 ----------
                        info = {}       # c -> (xl_ap, oht, oh)
                        pgroups = []    # per group: p tile AP [128, nch, H]
                        for gidx, (cids, xls) in enumerate(groups):
                            ng = len(cids)
                            lrc = wk.tile([128, MAXG, HID_], F32, tag="lrc",
                                          name="lrct")
                            for j, (c, xla) in enumerate(zip(cids, xls)):
                                if c == 0:
                                    oht = oh = None
                                else:
                                    oht = ohp.tile([128, 128], F32R,
                                                   tag=f"oht{c}", name=f"oht{c}t")
                                    nc.vector.tensor_scalar(
                                        oht[:], iota_sb[:],
                                        dst_sb[:, c:c + 1], None, ALU.is_equal)
                                    tp = pst.tile([128, 128], F32R, tag="ohtp")
                                    nc.tensor.transpose(tp[:], oht[:], id_sb[:])
                                    oh = ohp.tile([128, 128], F32R,
                                                  tag=f"oh{c}", name=f"oh{c}t")
                                    nc.scalar.copy(oh[:], tp[:])
                                info[c] = (xla, oht, oh)
                                ef = psp.tile([128, HID_], F32, tag="ef")
                                nc.tensor.matmul(
                                    ef[:], oh[:] if oh is not None else id_sb[:],
                                    xr_blk[:], start=True, stop=False)
                                nc.tensor.matmul(ef[:], id_sb[:], xla,
                                                 start=False, stop=True)
                                nc.scalar.activation(lrc[:, j, :], ef[:],
                                                     AF.Prelu, alpha=cfg.NEG)
                            # batched logits for the group
                            tmpc = wk.tile([128, MAXG, HID_], F32, tag="tmpc",
                                           name="tmpct")
                            attb = bc_sb[att_nm][:].rearrange(
                                "p (a f) -> p a f", a=1).broadcast_to(
                                [128, ng, HID_])
                            nc.vector.tensor_tensor(
                                tmpc[:, :ng, :], lrc[:, :ng, :], attb, ALU.mult)
                            lgc = wk.tile([128, MAXG, H_], F32, tag="lgc",
                                          name="lgct")
                            nc.vector.tensor_reduce(
                                lgc[:, :ng, :],
                                tmpc[:, :ng, :].rearrange(
                                    "p a (h d) -> p a h d", h=H_),
                                mybir.AxisListType.X, ALU.add)
                            pc = ohp.tile([128, MAXG, H_], F32R, tag=f"pg{gidx}",
                                          name=f"pg{gidx}t")
                            nc.scalar.activation(pc[:, :ng, :], lgc[:, :ng, :],
                                                 AF.Exp)
                            pgroups.append(pc)
                            for j, c in enumerate(cids):
                                _, oht, _ = info[c]
                                nc.tensor.matmul(
                                    denom_ps[:], oht[:] if oht is not None
                                    else id_sb[:], pc[:, j, :],
                                    start=(c == 0), stop=(c == NCH))

                        dsum = wk.tile([128, H_], F32, tag="ds")
                        nc.vector.tensor_scalar_add(dsum[:], denom_ps[:], 1e-16)
                        recip = blp.tile([128, H_], F32R, tag="rc")
                        nc.vector.reciprocal(recip[:], dsum[:])

                        # ---------- pass 2 ----------
                        for gidx, (cids, xls) in enumerate(groups):
                            ng = len(cids)
                            pc = pgroups[gidx]
                            rgc = psrg.tile([128, MAXG * H_], F32, tag="rg")
                            for j, c in enumerate(cids):
                                _, oht, oh = info[c]
                                nc.tensor.matmul(
                                    rgc[:, j * H_:(j + 1) * H_],
                                    oh[:] if oh is not None else id_sb[:],
                                    recip[:], start=True, stop=True)
                            alc = wk.tile([128, MAXG, H_], F32, tag="alc",
                                          name="alct")
                            nc.vector.tensor_tensor(
                                alc[:, :ng, :], pc[:, :ng, :],
                                rgc[:, :ng * H_].rearrange(
                                    "p (a h) -> p a h", h=H_), ALU.mult)
                            c0g = cids[0]
                            nc.sync.dma_start(
                                alpha_o[l][b, c0g:c0g + ng].rearrange(
                                    "c p h -> p c h"), alc[:, :ng, :])
                            scc = wk.tile([128, MAXG, HID_], F32R, tag="scc",
                                          name="scct")
                            for j, (c, xla) in enumerate(zip(cids, xls)):
                                pass
                            xg_in = (xl_self[:].rearrange(
                                "p (a f) -> p a f", a=1) if gidx == 0
                                else xg[gidx - 1][:, :ng, :])
                            nc.vector.tensor_tensor(
                                scc[:, :ng, :].rearrange(
                                    "p a (h d) -> p a h d", h=H_),
                                xg_in.rearrange("p a (h d) -> p a h d", h=H_),
                                alc[:, :ng, :].rearrange(
                                    "p a (h o) -> p a h o", o=1).broadcast_to(
                                    [128, ng, H_, D_]),
                                ALU.mult)
                            for j, c in enumerate(cids):
                                _, oht, _ = info[c]
                                nc.tensor.matmul(
                                    out_ps[:], oht[:] if oht is not None
                                    else id_sb[:], scc[:, j, :],
                                    start=(c == 0), stop=(c == NCH))

                        t1 = wk.tile([128, HID_], F32, tag="t1")
                        nc.vector.tensor_add(t1[:], out_ps[:], bc_sb[bias_nm][:])
                        t2 = wk.tile([128, HID_], F32, tag="t2")
                        nc.vector.tensor_add(t2[:], t1[:], hprev[:])
                        mn = wk.tile([128, HID_], F32, tag="mn")
                        nc.vector.tensor_scalar_min(mn[:], t2[:], 0.0)
                        ex = wk.tile([128, HID_], F32, tag="ex")
                        nc.scalar.activation(ex[:], mn[:], AF.Exp)
                        mx = wk.tile([128, HID_], F32, tag="mx")
                        nc.vector.tensor_scalar(mx[:], t2[:], 0.0, -1.0,
                                                ALU.max, ALU.add)
                        hout = blp.tile([128, HID_], F32R, tag="ho")
                        nc.vector.tensor_add(hout[:], ex[:], mx[:])
                        if l == 0:
                            nc.sync.dma_start(
                                h_nm[1][r0:r0 + 128, :].bitcast(F32R), hout[:])
                            h1T_t = blp.tile([128, 2, 128], F32R, tag="h1T")
                            for f in range(2):
                                tp2 = psep.tile([128, HID_], F32R, tag="ep",
                                                name="eptp")
                                nc.tensor.transpose(
                                    tp2[:, :128],
                                    hout[:, f * 128:(f + 1) * 128], id_sb[:])
                                nc.scalar.copy(h1T_t[:, f, :], tp2[:, :128])
                            for nm, dstd, bias in (("Wl1", xl_sl[1], "bl1_b"),
                                                   ("Wr1", xr_sl[1], "br1_b")):
                                ps = psep.tile([128, HID_], F32, tag="ep",
                                               name="epps")
                                for k in range(2):
                                    nc.tensor.matmul(
                                        ps[:], h1T_t[:, k, :], W_sb[nm][:, k, :],
                                        start=(k == 0), stop=(k == 1))
                                xo = wk.tile([128, HID_], F32, tag="xo1")
                                nc.vector.tensor_add(xo[:], ps[:], bc_sb[bias][:])
                                nc.sync.dma_start(dstd[r0:r0 + 128, :], xo[:])
                        else:
                            t3 = wk.tile([128, HID_], F32, tag="t3")
                            nc.vector.tensor_mul(t3[:], hout[:],
                                                 bc_sb["clfw_b"][:])
                            pr = wk.tile([128, 1], F32, tag="pr")
                            nc.vector.tensor_reduce(
                                pr[:], t3[:], mybir.AxisListType.X, ALU.add)
                            nc.sync.dma_start(preds_o[r0:r0 + 128, :], pr[:])

            edge_layer(0)
            nc.gpsimd.collective_compute(
                "AllGather", ALU.bypass, ins=[xl_sl[1][:]], outs=[xl_full[1][:]],
                replica_groups=[list(range(NCORES))])
            edge_layer(1)

    nc.compile()
    return nc


# ---------------------------------------------------------------- unshard

def unshard(cfg, results, meta):
    Etot = cfg.E + cfg.N
    preds = np.concatenate(
        [results[c]["preds"][:cfg.NLOC, 0] for c in range(NCORES)])
    preds = (preds + np.float32(meta["clf_b"])).astype(np.float32)

    alphas = []
    for l in range(2):
        af = np.zeros((Etot, cfg.H), np.float32)
        for c in range(NCORES):
            a = results[c][f"alpha{l}"]
            nodes = np.arange(cfg.NPAD)
            valid = nodes < cfg.NLOC
            sl = a[:, 0, :, :].reshape(cfg.NPAD, cfg.H)
            af[cfg.E + c * cfg.NLOC + nodes[valid]] = sl[valid]
            oid = meta["slot_oid"][c].reshape(-1)
            rnd = a[:, 1:, :, :].reshape(-1, cfg.H)
            m = oid >= 0
            af[oid[m]] = rnd[m]
        alphas.append(af)
    return preds, alphas[0], alphas[1]


# ---------------------------------------------------------------- entry

_CACHE = {}


def _run(inputs, trace=False, call_size=None):
    edge_index = np.asarray(inputs["edge_index"], np.int64)
    last_err = None
    sizes = (call_size,) if call_size else (8, 4, 2)
    for nch in (18, 20, 22, 26, 32):
        for cs in sizes:
            cfg = make_cfg(N, E, IN_DIM, nch, cs)
            try:
                in_maps, meta = prep_inputs(cfg, inputs)
            except ValueError as e:
                last_err = e
                continue
            key = (nch, cfg.CALL_CHUNKS, tuple(meta["bases"]))
            if key not in _CACHE:
                _CACHE[key] = build_program(cfg, meta["bases"])
            nc = _CACHE[key]
            res = run_bass_kernel_spmd(nc, in_maps, list(range(NCORES)),
                                       trace=trace)
            return cfg, meta, res
    raise RuntimeError(f"no feasible cfg found: {last_err}")


def kernel(**inputs):
    cfg, meta, res = _run(inputs)
    return unshard(cfg, res.results, meta)


# revision 7
# speedup vs baseline: 1.0624x; 1.0171x over previous
"""nn_GATv2Net kernel for 8 TRN2 NeuronCores (self-contained).

kernel(**inputs) takes the FULL unsharded inputs of the reference
(x [50000,1280], edge_index [2,800000] plus weights) and returns
(preds [50000], alpha0 [850000,4], alpha1 [850000,4]) as float32, matching
reference.reference().

Distribution: nodes block-partitioned across the 8 cores; each edge is owned
by the core holding its dst node. Edges are grouped into per-core blocks of
128 consecutive local dst nodes and sorted by src within the block. Per-edge
src features are fetched with SWDGE dma_gather from an AllGather-replicated
xl table; segment softmax + aggregation run as one-hot f32r matmuls in PSUM.
"""
import dataclasses
import numpy as np

import concourse.bacc as bacc
import concourse.mybir as mybir
from concourse import tile
from concourse.bass_utils import run_bass_kernel_spmd

F32 = mybir.dt.float32
F32R = mybir.dt.float32r
I16 = mybir.dt.int16
AF = mybir.ActivationFunctionType
ALU = mybir.AluOpType

NCORES = 8
N, E, IN_DIM, HID, H, D = 50000, 800000, 1280, 256, 4, 64
NEG = 0.2


@dataclasses.dataclass
class Cfg:
    N: int = N
    E: int = E
    IN_DIM: int = IN_DIM
    HID: int = HID
    H: int = H
    D: int = D
    NEG: float = NEG
    NLOC: int = N // NCORES
    NB: int = (N // NCORES + 127) // 128
    NPAD: int = ((N // NCORES + 127) // 128) * 128
    NCH: int = 18
    CALL_CHUNKS: tuple = (8, 8, 2)

    @property
    def NGPAD(self):
        return NCORES * self.NPAD

    @property
    def KIN(self):
        return self.IN_DIM // 128


def make_cfg(N_, E_, IN_DIM_, nch, call_size=8):
    nloc = N_ // NCORES
    nb = (nloc + 127) // 128
    cc = []
    left = nch
    while left > 0:
        cc.append(min(call_size, left))
        left -= cc[-1]
    return Cfg(N=N_, E=E_, IN_DIM=IN_DIM_, NLOC=nloc, NB=nb, NPAD=nb * 128,
               NCH=nch, CALL_CHUNKS=tuple(cc))


# ---------------------------------------------------------------- host prep

def wrap_idx_flat(idx):
    """[n] int (n % 16 == 0) -> [128, n//16] int16 SWDGE wrapped layout."""
    n = idx.shape[0]
    a = idx.astype(np.int16).reshape(n // 16, 16)
    return np.tile(a.T, (8, 1))


def prep_graph(cfg, edge_index):
    src = np.asarray(edge_index[0], np.int64)
    dst = np.asarray(edge_index[1], np.int64)

    core = dst // cfg.NLOC
    ldst = dst - core * cfg.NLOC
    blk = ldst // 128
    drel = ldst % 128
    sg = (src // cfg.NLOC) * cfg.NPAD + (src % cfg.NLOC)

    order = np.lexsort((sg, blk, core))
    core_s, blk_s, drel_s, sg_s = core[order], blk[order], drel[order], sg[order]

    NCH, NB = cfg.NCH, cfg.NB
    cap = NCH * 128
    key = core_s * NB + blk_s
    bounds = np.searchsorted(key, np.arange(NCORES * NB + 1))
    counts = np.diff(bounds)
    if counts.max() > cap:
        raise ValueError(f"block overflow: {counts.max()} > {cap}")

    slot_sg = np.zeros((NCORES * NB, cap), np.int64)
    slot_dr = np.full((NCORES * NB, cap), -1.0, np.float32)
    slot_oid = np.full((NCORES * NB, cap), -1, np.int64)
    ar = np.arange(cap)
    mask = ar[None, :] < counts[:, None]
    idx_flat = np.nonzero(mask)
    slot_sg[idx_flat] = sg_s
    slot_dr[idx_flat] = drel_s.astype(np.float32)
    slot_oid[idx_flat] = order
    lastv = slot_sg[np.arange(NCORES * NB), np.maximum(counts - 1, 0)]
    slot_sg[~mask] = np.repeat(lastv, cap - counts)
    slot_sg[counts == 0] = 0

    slot_sg = slot_sg.reshape(NCORES, NB, NCH, 128)
    slot_dr = slot_dr.reshape(NCORES, NB, NCH, 128)
    slot_oid = slot_oid.reshape(NCORES, NB, NCH, 128)

    bases, calls = [], []
    c0 = 0
    for nch in cfg.CALL_CHUNKS:
        seg = slot_sg[:, :, c0:c0 + nch, :]
        b = int(seg.min())
        span = int(seg.max()) - b
        if span > 32767:
            raise ValueError(f"span {span} > 32767 (chunks {c0}..{c0+nch})")
        bases.append(b)
        calls.append((c0, nch))
        c0 += nch

    idx16 = np.zeros((NCORES, NB, 128, NCH * 8), np.int16)
    for (c0, nch), b in zip(calls, bases):
        rel = slot_sg[:, :, c0:c0 + nch, :] - b
        flat = rel.reshape(NCORES, NB, nch * 128)
        for ci in range(NCORES):
            for bi in range(NB):
                idx16[ci, bi, :, c0 * 8:(c0 + nch) * 8] = wrap_idx_flat(flat[ci, bi])

    dstrel = np.zeros((NCORES, NB, NCH + 1, 128), np.float32)
    selfvalid = (np.arange(cfg.NPAD) < cfg.NLOC).reshape(NB, 128)
    dstrel[:, :, 0, :] = np.where(selfvalid[None], np.arange(128)[None, None], -1.0)
    dstrel[:, :, 1:, :] = slot_dr

    nchb = np.maximum(
        1, -(-counts.reshape(NCORES, NB).max(axis=0) // 128)).astype(int)
    return dict(idx16=idx16, dstrel=dstrel, bases=bases, calls=calls,
                slot_oid=slot_oid, nchb=nchb)


def prep_inputs(cfg, inputs):
    x = np.asarray(inputs["x"], np.float32)
    g = prep_graph(cfg, np.asarray(inputs["edge_index"], np.int64))

    iota_row = np.tile(np.arange(128, dtype=np.float32), (128, 1))
    ident = np.eye(128, dtype=np.float32)

    def bcast(v):
        return np.tile(np.asarray(v, np.float32).reshape(1, -1), (128, 1))

    shared = dict(
        enc_W=np.ascontiguousarray(inputs["enc_W"], np.float32),
        Wl0=np.ascontiguousarray(inputs["Wl0"], np.float32),
        Wr0=np.ascontiguousarray(inputs["Wr0"], np.float32),
        Wl1=np.ascontiguousarray(inputs["Wl1"], np.float32),
        Wr1=np.ascontiguousarray(inputs["Wr1"], np.float32),
        bl0_b=bcast(np.asarray(inputs["enc_b"], np.float32)
                    @ np.asarray(inputs["Wl0"], np.float32)
                    + np.asarray(inputs["bl0"], np.float32)),
        br0_b=bcast(np.asarray(inputs["enc_b"], np.float32)
                    @ np.asarray(inputs["Wr0"], np.float32)
                    + np.asarray(inputs["br0"], np.float32)),
        bl1_b=bcast(inputs["bl1"]), br1_b=bcast(inputs["br1"]),
        bias0_b=bcast(np.asarray(inputs["bias0"], np.float32)
                      + np.asarray(inputs["enc_b"], np.float32)),
        bias1_b=bcast(inputs["bias1"]),
        att0_b=bcast(np.asarray(inputs["att0"], np.float32).reshape(-1)),
        att1_b=bcast(np.asarray(inputs["att1"], np.float32).reshape(-1)),
        clfw_b=bcast(np.asarray(inputs["clf_W"], np.float32).reshape(-1)),
        iota_row=iota_row, ident=ident,
    )
    clf_b = float(np.asarray(inputs["clf_b"]).reshape(-1)[0])

    in_maps = []
    for c in range(NCORES):
        xc = x[c * cfg.NLOC:(c + 1) * cfg.NLOC]
        xT = np.zeros((cfg.IN_DIM, cfg.NPAD), np.float32)
        xT[:, :cfg.NLOC] = xc.T
        m = dict(shared)
        m["xT"] = xT
        m["idx16"] = g["idx16"][c]
        m["dstrel"] = g["dstrel"][c]
        in_maps.append(m)

    meta = dict(bases=g["bases"], calls=g["calls"], slot_oid=g["slot_oid"],
                clf_b=clf_b, nchb=g["nchb"])
    return in_maps, meta


# ---------------------------------------------------------------- builder

def build_program(cfg, bases, nchb=None):
    HID_, H_, D_ = cfg.HID, cfg.H, cfg.D
    NB, NCH, NPAD, NGPAD = cfg.NB, cfg.NCH, cfg.NPAD, cfg.NGPAD
    KIN = cfg.KIN
    CALLS = []
    c0 = 0
    for nch in cfg.CALL_CHUNKS:
        CALLS.append((c0, nch))
        c0 += nch
    MAXG = max(cfg.CALL_CHUNKS)
    if nchb is None:
        nchb = [NCH] * cfg.NB

    nc = bacc.Bacc("TRN2", target_bir_lowering=False, debug=False,
                   num_devices=NCORES)

    xT = nc.declare_dram_parameter("xT", [cfg.IN_DIM, NPAD], F32, isOutput=False)
    idx16 = nc.declare_dram_parameter("idx16", [NB, 128, NCH * 8], I16, isOutput=False)
    dstrel = nc.declare_dram_parameter("dstrel", [NB, NCH + 1, 128], F32, isOutput=False)
    enc_W = nc.declare_dram_parameter("enc_W", [cfg.IN_DIM, HID_], F32, isOutput=False)
    Ws = {}
    for nm in ("Wl0", "Wr0", "Wl1", "Wr1"):
        Ws[nm] = nc.declare_dram_parameter(nm, [HID_, HID_], F32, isOutput=False)
    bc = {}
    for nm in ("bl0_b", "br0_b", "bl1_b", "br1_b", "bias0_b",
               "bias1_b", "att0_b", "att1_b", "clfw_b"):
        bc[nm] = nc.declare_dram_parameter(nm, [128, HID_], F32, isOutput=False)
    iota_d = nc.declare_dram_parameter("iota_row", [128, 128], F32, isOutput=False)
    ident_d = nc.declare_dram_parameter("ident", [128, 128], F32, isOutput=False)

    preds_o = nc.declare_dram_parameter("preds", [NPAD, 1], F32, isOutput=True)
    alpha_o = [
        nc.declare_dram_parameter(f"alpha{l}", [NB, NCH + 1, 128, H_], F32,
                                  isOutput=True)
        for l in range(2)
    ]

    def dram(name, shape):
        return nc.dram_tensor(name, shape, F32)

    h_nm = [dram("h0_nm", [NPAD, HID_]), dram("h1_nm", [NPAD, HID_])]
    xl_sl = [dram("xl0_sl", [NPAD, HID_]), dram("xl1_sl", [NPAD, HID_])]
    xr_sl = [dram("xr0_sl", [NPAD, HID_]), dram("xr1_sl", [NPAD, HID_])]
    xl_full = [
        nc.dram_tensor("xl0_full", [NGPAD, HID_], F32, addr_space="Shared"),
        nc.dram_tensor("xl1_full", [NGPAD, HID_], F32, addr_space="Shared"),
    ]

    with tile.TileContext(nc) as tc, nc.allow_low_precision(
            reason="f32r tiles are bit-identical to f32"):
        with tc.tile_pool(name="const", bufs=1) as constp:
            encW_sb = constp.tile([128, KIN, HID_], F32R)
            nc.sync.dma_start(
                encW_sb[:],
                enc_W[:].rearrange("(k p) f -> p k f", p=128).bitcast(F32R))
            W_sb = {}
            for nm in Ws:
                W_sb[nm] = constp.tile([128, 2, HID_], F32R, tag=f"W{nm}",
                                       name=f"W{nm}sb")
                nc.sync.dma_start(
                    W_sb[nm][:],
                    Ws[nm][:].rearrange("(k p) f -> p k f", p=128).bitcast(F32R))
            bc_sb = {}
            for nm in bc:
                bc_sb[nm] = constp.tile([128, HID_], F32, tag=f"b{nm}",
                                        name=f"bc{nm}sb")
                nc.sync.dma_start(bc_sb[nm][:], bc[nm][:])
            iota_sb = constp.tile([128, 128], F32)
            nc.sync.dma_start(iota_sb[:], iota_d[:])
            id_sb = constp.tile([128, 128], F32R)
            nc.sync.dma_start(id_sb[:], ident_d[:].bitcast(F32R))

            # ---- Phase A: encoder + layer-0 transforms
            with (
                tc.tile_pool(name="enc_in", bufs=3) as enc_in,
                tc.tile_pool(name="enc_ps", bufs=2, space="PSUM") as enc_ps,
                tc.tile_pool(name="enc_out", bufs=3) as enc_out,
            ):
                nchunks = NPAD // 512 + (1 if NPAD % 512 else 0)
                for ch in range(nchunks):
                    n0 = ch * 512
                    nn = min(512, NPAD - n0)
                    xt_t = enc_in.tile([128, KIN, nn], F32R, tag="xt")
                    nc.sync.dma_start(
                        xt_t[:],
                        xT[:, n0:n0 + nn].rearrange(
                            "(k p) n -> p k n", p=128).bitcast(F32R))
                    h0T_sb = enc_out.tile([128, 2, nn], F32R, tag="h0T")
                    for f in range(2):
                        ps = enc_ps.tile([128, 512], F32, tag="hps")
                        for k in range(KIN):
                            nc.tensor.matmul(
                                ps[:, :nn], encW_sb[:, k, f * 128:(f + 1) * 128],
                                xt_t[:, k, :], start=(k == 0), stop=(k == KIN - 1))
                        nc.scalar.copy(h0T_sb[:, f, :], ps[:, :nn])
                    for sb in range(nn // 128):
                        nsub = n0 + sb * 128
                        h0_t = enc_out.tile([128, HID_], F32, tag="h0nm")
                        for f in range(2):
                            tp = enc_ps.tile([128, 128], F32R, tag="tp")
                            nc.tensor.transpose(
                                tp[:], h0T_sb[:, f, sb * 128:(sb + 1) * 128],
                                id_sb[:])
                            nc.scalar.copy(h0_t[:, f * 128:(f + 1) * 128], tp[:])
                        nc.sync.dma_start(h_nm[0][nsub:nsub + 128, :], h0_t[:])
                        for nm, dstd, bias in (("Wl0", xl_sl[0], "bl0_b"),
                                               ("Wr0", xr_sl[0], "br0_b")):
                            ps = enc_ps.tile([128, HID_], F32, tag="xps")
                            for k in range(2):
                                nc.tensor.matmul(
                                    ps[:], h0T_sb[:, k, sb * 128:(sb + 1) * 128],
                                    W_sb[nm][:, k, :], start=(k == 0),
                                    stop=(k == 1))
                            xo = enc_out.tile([128, HID_], F32, tag="xo")
                            nc.vector.tensor_add(xo[:], ps[:], bc_sb[bias][:])
                            nc.sync.dma_start(dstd[nsub:nsub + 128, :], xo[:])

            nc.gpsimd.collective_compute(
                "AllGather", ALU.bypass, ins=[xl_sl[0][:]], outs=[xl_full[0][:]],
                replica_groups=[list(range(NCORES))])

            def edge_layer(l):
                att_nm = f"att{l}_b"
                bias_nm = f"bias{l}_b"
                with (
                    tc.tile_pool(name=f"gin{l}", bufs=2) as gin,
                    tc.tile_pool(name=f"oh{l}", bufs=2) as ohp,
                    tc.tile_pool(name=f"wk{l}", bufs=2) as wk,
                    tc.tile_pool(name=f"bl{l}", bufs=3) as blp,
                    tc.tile_pool(name=f"ps{l}", bufs=1, space="PSUM") as psp,
                    tc.tile_pool(name=f"pst{l}", bufs=1, space="PSUM") as pst,
                    tc.tile_pool(name=f"acc{l}", bufs=2, space="PSUM") as accp,
                    tc.tile_pool(name=f"psr{l}", bufs=1, space="PSUM") as psrg,
                    tc.tile_pool(name=f"pse{l}", bufs=1, space="PSUM") as psep,
                ):
                    for b in range(NB):
                        r0 = b * 128
                        idx_sb = blp.tile([128, NCH * 8], I16, tag="idx")
                        nc.sync.dma_start(idx_sb[:], idx16[b])
                        dst_sb = blp.tile([128, NCH + 1], F32, tag="dst")
                        nc.sync.dma_start(
                            dst_sb[:], dstrel[b].rearrange("c p -> p c"))
                        xr_blk = blp.tile([128, HID_], F32R, tag="xr")
                        nc.sync.dma_start(
                            xr_blk[:], xr_sl[l][r0:r0 + 128, :].bitcast(F32R))
                        hprev = blp.tile([128, HID_], F32, tag="hp")
                        nc.sync.dma_start(hprev[:], h_nm[l][r0:r0 + 128, :])
                        xl_self = gin.tile([128, HID_], F32R, tag="xls")
                        nc.sync.dma_start(
                            xl_self[:], xl_sl[l][r0:r0 + 128, :].bitcast(F32R))

                        denom_ps = accp.tile([128, H_], F32, tag="denom")
                        out_ps = accp.tile([128, HID_], F32, tag="out")

                        # gather calls
                        nb_ch = nchb[b]
                        last_c = nb_ch  # last global chunk id (self=0)
                        xg = []
                        bcalls = []
                        for gi, (c0, nch) in enumerate(CALLS):
                            nch_eff = max(0, min(nch, nb_ch - c0))
                            if nch_eff == 0:
                                xg.append(None)
                                continue
                            bcalls.append((gi, c0, nch_eff))
                            t = gin.tile([128, MAXG, HID_], F32R, tag=f"g{gi}",
                                         name=f"g{gi}t")
                            base = bases[gi]
                            wrows = min(32768, NGPAD - base)
                            nc.gpsimd.dma_gather(
                                t[:, :nch_eff, :],
                                xl_full[l][base:base + wrows, :].bitcast(F32R),
                                idx_sb[:, c0 * 8:(c0 + nch_eff) * 8],
                                nch_eff * 128, nch_eff * 128, HID_)
                            xg.append(t)

                        # groups: (chunk ids, xl aps). group 0 = self loops.
                        groups = [([0], [xl_self[:]])]
                        for gi, c0, nch_eff in bcalls:
                            groups.append((
                                [c0 + 1 + j for j in range(nch_eff)],
                                [xg[gi][:, j, :] for j in range(nch_eff)],
                            ))

                        # ---------- pass 1 ----------
                        info = {}       # c -> (xl_ap, oht, oh)
                        pgroups = []    # per group: p tile AP [128, nch, H]
                        for gidx, (cids, xls) in enumerate(groups):
                            ng = len(cids)
                            lrc = wk.tile([128, MAXG, HID_], F32, tag="lrc",
                                          name="lrct")
                            for j, (c, xla) in enumerate(zip(cids, xls)):
                                if c == 0:
                                    oht = oh = None
                                else:
                                    oht = ohp.tile([128, 128], F32R,
                                                   tag=f"oht{c}", name=f"oht{c}t")
                                    nc.vector.tensor_scalar(
                                        oht[:], iota_sb[:],
                                        dst_sb[:, c:c + 1], None, ALU.is_equal)
                                    tp = pst.tile([128, 128], F32R, tag="ohtp")
                                    nc.tensor.transpose(tp[:], oht[:], id_sb[:])
                                    oh = ohp.tile([128, 128], F32R,
                                                  tag=f"oh{c}", name=f"oh{c}t")
                                    nc.scalar.copy(oh[:], tp[:])
                                info[c] = (xla, oht, oh)
                                ef = psp.tile([128, HID_], F32, tag="ef")
                                nc.tensor.matmul(
                                    ef[:], oh[:] if oh is not None else id_sb[:],
                                    xr_blk[:], start=True, stop=False)
                                nc.tensor.matmul(ef[:], id_sb[:], xla,
                                                 start=False, stop=True)
                                nc.scalar.activation(lrc[:, j, :], ef[:],
                                                     AF.Prelu, alpha=cfg.NEG)
                            # batched logits for the group
                            tmpc = wk.tile([128, MAXG, HID_], F32, tag="tmpc",
                                           name="tmpct")
                            attb = bc_sb[att_nm][:].rearrange(
                                "p (a f) -> p a f", a=1).broadcast_to(
                                [128, ng, HID_])
                            nc.vector.tensor_tensor(
                                tmpc[:, :ng, :], lrc[:, :ng, :], attb, ALU.mult)
                            lgc = wk.tile([128, MAXG, H_], F32, tag="lgc",
                                          name="lgct")
                            nc.vector.tensor_reduce(
                                lgc[:, :ng, :],
                                tmpc[:, :ng, :].rearrange(
                                    "p a (h d) -> p a h d", h=H_),
                                mybir.AxisListType.X, ALU.add)
                            pc = ohp.tile([128, MAXG, H_], F32R, tag=f"pg{gidx}",
                                          name=f"pg{gidx}t")
                            nc.scalar.activation(pc[:, :ng, :], lgc[:, :ng, :],
                                                 AF.Exp)
                            pgroups.append(pc)
                            for j, c in enumerate(cids):
                                _, oht, _ = info[c]
                                nc.tensor.matmul(
                                    denom_ps[:], oht[:] if oht is not None
                                    else id_sb[:], pc[:, j, :],
                                    start=(c == 0), stop=(c == last_c))

                        dsum = wk.tile([128, H_], F32, tag="ds")
                        nc.vector.tensor_scalar_add(dsum[:], denom_ps[:], 1e-16)
                        recip = blp.tile([128, H_], F32R, tag="rc")
                        nc.vector.reciprocal(recip[:], dsum[:])

                        # ---------- pass 2 ----------
                        for gidx, (cids, xls) in enumerate(groups):
                            ng = len(cids)
                            pc = pgroups[gidx]
                            rgc = psrg.tile([128, MAXG * H_], F32, tag="rg")
                            for j, c in enumerate(cids):
                                _, oht, oh = info[c]
                                nc.tensor.matmul(
                                    rgc[:, j * H_:(j + 1) * H_],
                                    oh[:] if oh is not None else id_sb[:],
                                    recip[:], start=True, stop=True)
                            alc = wk.tile([128, MAXG, H_], F32, tag="alc",
                                          name="alct")
                            nc.vector.tensor_tensor(
                                alc[:, :ng, :], pc[:, :ng, :],
                                rgc[:, :ng * H_].rearrange(
                                    "p (a h) -> p a h", h=H_), ALU.mult)
                            c0g = cids[0]
                            nc.sync.dma_start(
                                alpha_o[l][b, c0g:c0g + ng].rearrange(
                                    "c p h -> p c h"), alc[:, :ng, :])
                            scc = wk.tile([128, MAXG, HID_], F32R, tag="scc",
                                          name="scct")
                            for j, (c, xla) in enumerate(zip(cids, xls)):
                                pass
                            xg_in = (xl_self[:].rearrange(
                                "p (a f) -> p a f", a=1) if gidx == 0
                                else xg[bcalls[gidx - 1][0]][:, :ng, :])
                            nc.vector.tensor_tensor(
                                scc[:, :ng, :].rearrange(
                                    "p a (h d) -> p a h d", h=H_),
                                xg_in.rearrange("p a (h d) -> p a h d", h=H_),
                                alc[:, :ng, :].rearrange(
                                    "p a (h o) -> p a h o", o=1).broadcast_to(
                                    [128, ng, H_, D_]),
                                ALU.mult)
                            for j, c in enumerate(cids):
                                _, oht, _ = info[c]
                                nc.tensor.matmul(
                                    out_ps[:], oht[:] if oht is not None
                                    else id_sb[:], scc[:, j, :],
                                    start=(c == 0), stop=(c == last_c))

                        t1 = wk.tile([128, HID_], F32, tag="t1")
                        nc.vector.tensor_add(t1[:], out_ps[:], bc_sb[bias_nm][:])
                        t2 = wk.tile([128, HID_], F32, tag="t2")
                        nc.vector.tensor_add(t2[:], t1[:], hprev[:])
                        mn = wk.tile([128, HID_], F32, tag="mn")
                        nc.vector.tensor_scalar_min(mn[:], t2[:], 0.0)
                        ex = wk.tile([128, HID_], F32, tag="ex")
                        nc.scalar.activation(ex[:], mn[:], AF.Exp)
                        mx = wk.tile([128, HID_], F32, tag="mx")
                        nc.vector.tensor_scalar(mx[:], t2[:], 0.0, -1.0,
                                                ALU.max, ALU.add)
                        hout = blp.tile([128, HID_], F32R, tag="ho")
                        nc.vector.tensor_add(hout[:], ex[:], mx[:])
                        if l == 0:
                            nc.sync.dma_start(
                                h_nm[1][r0:r0 + 128, :].bitcast(F32R), hout[:])
                            h1T_t = blp.tile([128, 2, 128], F32R, tag="h1T")
                            for f in range(2):
                                tp2 = psep.tile([128, HID_], F32R, tag="ep",
                                                name="eptp")
                                nc.tensor.transpose(
                                    tp2[:, :128],
                                    hout[:, f * 128:(f + 1) * 128], id_sb[:])
                                nc.scalar.copy(h1T_t[:, f, :], tp2[:, :128])
                            for nm, dstd, bias in (("Wl1", xl_sl[1], "bl1_b"),
                                                   ("Wr1", xr_sl[1], "br1_b")):
                                ps = psep.tile([128, HID_], F32, tag="ep",
                                               name="epps")
                                for k in range(2):
                                    nc.tensor.matmul(
                                        ps[:], h1T_t[:, k, :], W_sb[nm][:, k, :],
                                        start=(k == 0), stop=(k == 1))
                                xo = wk.tile([128, HID_], F32, tag="xo1")
                                nc.vector.tensor_add(xo[:], ps[:], bc_sb[bias][:])
                                nc.sync.dma_start(dstd[r0:r0 + 128, :], xo[:])
                        else:
                            t3 = wk.tile([128, HID_], F32, tag="t3")
                            nc.vector.tensor_mul(t3[:], hout[:],
                                                 bc_sb["clfw_b"][:])
                            pr = wk.tile([128, 1], F32, tag="pr")
                            nc.vector.tensor_reduce(
                                pr[:], t3[:], mybir.AxisListType.X, ALU.add)
                            nc.sync.dma_start(preds_o[r0:r0 + 128, :], pr[:])

            edge_layer(0)
            nc.gpsimd.collective_compute(
                "AllGather", ALU.bypass, ins=[xl_sl[1][:]], outs=[xl_full[1][:]],
                replica_groups=[list(range(NCORES))])
            edge_layer(1)

    nc.compile()
    return nc


# ---------------------------------------------------------------- unshard

def unshard(cfg, results, meta):
    Etot = cfg.E + cfg.N
    preds = np.concatenate(
        [results[c]["preds"][:cfg.NLOC, 0] for c in range(NCORES)])
    preds = (preds + np.float32(meta["clf_b"])).astype(np.float32)

    alphas = []
    for l in range(2):
        af = np.zeros((Etot, cfg.H), np.float32)
        for c in range(NCORES):
            a = results[c][f"alpha{l}"]
            nodes = np.arange(cfg.NPAD)
            valid = nodes < cfg.NLOC
            sl = a[:, 0, :, :].reshape(cfg.NPAD, cfg.H)
            af[cfg.E + c * cfg.NLOC + nodes[valid]] = sl[valid]
            oid = meta["slot_oid"][c].reshape(-1)
            rnd = a[:, 1:, :, :].reshape(-1, cfg.H)
            m = oid >= 0
            af[oid[m]] = rnd[m]
        alphas.append(af)
    return preds, alphas[0], alphas[1]


# ---------------------------------------------------------------- entry

_CACHE = {}


def _run(inputs, trace=False, call_size=None):
    edge_index = np.asarray(inputs["edge_index"], np.int64)
    last_err = None
    sizes = (call_size,) if call_size else (8, 4, 2)
    for nch in (18, 20, 22, 26, 32):
        for cs in sizes:
            cfg = make_cfg(N, E, IN_DIM, nch, cs)
            try:
                in_maps, meta = prep_inputs(cfg, inputs)
            except ValueError as e:
                last_err = e
                continue
            key = (nch, cfg.CALL_CHUNKS, tuple(meta["bases"]),
                   tuple(meta["nchb"]))
            if key not in _CACHE:
                _CACHE[key] = build_program(cfg, meta["bases"], meta["nchb"])
            nc = _CACHE[key]
            res = run_bass_kernel_spmd(nc, in_maps, list(range(NCORES)),
                                       trace=trace)
            return cfg, meta, res
    raise RuntimeError(f"no feasible cfg found: {last_err}")


def kernel(**inputs):
    cfg, meta, res = _run(inputs)
    return unshard(cfg, res.results, meta)
